# revision 6
# baseline (speedup 1.0000x reference)
"""Greedy autoregressive LSTM decoder on 8 TRN2 NeuronCores.

Strategy: vocab-shard the hidden->vocab projection and the embedding-table
argmax across the 8 cores (4000 vocab rows each, W_out shard resident in
SBUF); replicate the small LSTM weights and redundantly compute the LSTM
cell on every core. Each step every core computes its local logits shard
and local (max, argmax); an 8-core AllGather of the 64 (val, idx) pairs
resolves the global argmax; each core then gathers the winning embedding
rows from its own full copy of the table in DRAM (indirect DMA) and feeds
them back. Logit shards stream to DRAM as they are produced and the host
concatenates along vocab.
"""

import sys

sys.path.insert(0, "/opt/trn_rl_repo")

import numpy as np

import concourse.bacc as bacc
import concourse.bass as bass
import concourse.tile as tile
from concourse import bass_utils, mybir

F32 = mybir.dt.float32
I32 = mybir.dt.int32
U32 = mybir.dt.uint32

B = 64          # batch
E = 512         # embed
H = 512         # hidden
V = 32000       # vocab
NCORES = 8
VL = V // NCORES  # vocab shard per core
GQ = 1024       # gate-psum half width (i,f | g,o)
LQ = 1000       # logits quarter width


def build(T: int, stage: int = 4):
    nc = bacc.Bacc("TRN2", target_bir_lowering=False, debug=False,
                   num_devices=NCORES)

    # ---- kernel I/O ----
    d_wcat = nc.dram_tensor("wcat", [128, 8, 2048], F32, kind="ExternalInput")
    d_bgate = nc.dram_tensor("bgate", [1, 2048], F32, kind="ExternalInput")
    d_wout = nc.dram_tensor("wout", [128, 4, VL], F32, kind="ExternalInput")
    d_bout = nc.dram_tensor("bout", [B, VL], F32, kind="ExternalInput")
    d_emb = nc.dram_tensor("emb", [V, E], F32, kind="ExternalInput")
    d_h0t = nc.dram_tensor("h0t", [128, 4, B], F32, kind="ExternalInput")
    d_x0t = nc.dram_tensor("x0t", [128, 4, B], F32, kind="ExternalInput")
    d_ones = nc.dram_tensor("ones", [1, B], F32, kind="ExternalInput")
    d_ident = nc.dram_tensor("ident", [128, 128], F32, kind="ExternalInput")
    d_iota8 = nc.dram_tensor("iota8", [B, 8], F32, kind="ExternalInput")
    d_vbase = nc.dram_tensor("myvbase", [B, 1], F32, kind="ExternalInput")
    d_out = nc.dram_tensor("logits", [B, T, VL], F32, kind="ExternalOutput")
    out_ap = d_out.ap()

    with tile.TileContext(nc) as tc:
        with tc.tile_pool(name="w", bufs=1) as wp, \
             tc.tile_pool(name="s", bufs=2) as sp, \
             tc.tile_pool(name="ps", bufs=2, space="PSUM") as pp, \
             tc.tile_pool(name="dr", bufs=2, space="DRAM") as dp:

            # ---- preload weights/constants ----
            wcat = wp.tile([128, 8, 2048], F32)
            wout = wp.tile([128, 4, VL], F32)
            bgate = wp.tile([1, 2048], F32)
            bout = wp.tile([B, VL], F32)
            ones = wp.tile([1, B], F32)
            ident = wp.tile([128, 128], F32)
            iota8 = wp.tile([B, 8], F32)
            vbase = wp.tile([B, 1], F32)
            for dst, src in ((wcat, d_wcat), (wout, d_wout), (bgate, d_bgate),
                             (bout, d_bout), (ones, d_ones), (ident, d_ident),
                             (iota8, d_iota8), (vbase, d_vbase)):
                nc.sync.dma_start(out=dst[:], in_=src.ap()[:])

            # ---- initial state ----
            hT = sp.tile([128, 4, B], F32, tag="hT", bufs=2)
            xT = sp.tile([128, 4, B], F32, tag="xT", bufs=2)
            c = sp.tile([B, H], F32, tag="c", bufs=2)
            nc.sync.dma_start(out=hT[:], in_=d_h0t.ap()[:])
            nc.sync.dma_start(out=xT[:], in_=d_x0t.ap()[:])
            nc.vector.memset(c[:], 0.0)

            for t in range(T):
                # ---- LSTM gates:  z = [x, h],  gates = z @ [Wih.T; Whh.T] + b
                # two halves: cols 0..1023 = (i, f), 1024..2047 = (g, o)
                gact = []  # sigmoid(i), sigmoid(f), tanh(g), sigmoid(o)
                for half in range(2):
                    psg = pp.tile([64, GQ], F32, tag="g", bufs=2)
                    for k in range(8):
                        zk = xT[:, k, :] if k < 4 else hT[:, k - 4, :]
                        for n2 in range(2):
                            nc.tensor.matmul(
                                out=psg[:, n2 * 512:(n2 + 1) * 512],
                                lhsT=zk,
                                rhs=wcat[:, k, half * GQ + n2 * 512:
                                         half * GQ + (n2 + 1) * 512],
                                start=(k == 0), stop=False)
                    for n2 in range(2):
                        nc.tensor.matmul(
                            out=psg[:, n2 * 512:(n2 + 1) * 512],
                            lhsT=ones[:, :],
                            rhs=bgate[:, half * GQ + n2 * 512:
                                      half * GQ + (n2 + 1) * 512],
                            start=False, stop=True)
                    for n2 in range(2):
                        act = sp.tile([B, 512], F32, tag=f"ga{half}{n2}", bufs=1)
                        fn = (mybir.ActivationFunctionType.Tanh
                              if (half, n2) == (1, 0)
                              else mybir.ActivationFunctionType.Sigmoid)
                        nc.scalar.activation(act[:], psg[:, n2 * 512:(n2 + 1) * 512], fn)
                        gact.append(act)

                si, sf, tg, so = gact
                # c' = sf * c + si * tg ;  h = so * tanh(c')
                t1 = sp.tile([B, H], F32, tag="t1", bufs=1)
                nc.vector.tensor_tensor(out=t1[:], in0=si[:], in1=tg[:],
                                        op=mybir.AluOpType.mult)
                c_new = sp.tile([B, H], F32, tag="c", bufs=2)
                nc.vector.tensor_tensor(out=c_new[:], in0=sf[:], in1=c[:],
                                        op=mybir.AluOpType.mult)
                nc.vector.tensor_tensor(out=c_new[:], in0=c_new[:], in1=t1[:],
                                        op=mybir.AluOpType.add)
                tc_ = sp.tile([B, H], F32, tag="tc", bufs=1)
                nc.scalar.activation(tc_[:], c_new[:],
                                     mybir.ActivationFunctionType.Tanh)
                h_new = sp.tile([B, H], F32, tag="h", bufs=1)
                nc.vector.tensor_tensor(out=h_new[:], in0=so[:], in1=tc_[:],
                                        op=mybir.AluOpType.mult)

                # ---- transpose h -> hT tiles for next matmuls
                hT_new = sp.tile([128, 4, B], F32, tag="hT", bufs=2)
                pst = pp.tile([128, 4 * B], F32, tag="g", bufs=2)
                for k in range(4):
                    nc.tensor.transpose(out=pst[:, k * B:(k + 1) * B],
                                        in_=h_new[:, k * 128:(k + 1) * 128],
                                        identity=ident[0:B, 0:B])
                nc.scalar.activation(hT_new[:].rearrange("p k b -> p (k b)"),
                                     pst[:],
                                     mybir.ActivationFunctionType.Copy)

                # ---- logits shard:  lg = h @ Wout_loc.T + bout_loc   [B, VL]
                lg = sp.tile([B, VL], F32, tag="lg", bufs=1)
                mq = sp.tile([B, 4], F32, tag="mq", bufs=2)
                for q in range(4):
                    psl = pp.tile([64, GQ], F32, tag="l", bufs=2)
                    for k in range(4):
                        for n2 in range(2):
                            nc.tensor.matmul(
                                out=psl[:, n2 * 512:n2 * 512 + 500],
                                lhsT=hT_new[:, k, :],
                                rhs=wout[:, k, q * LQ + n2 * 500:
                                         q * LQ + (n2 + 1) * 500],
                                start=(k == 0), stop=(k == 3))
                    # bias add + copy to SBUF, then per-quarter max
                    for n2 in range(2):
                        nc.vector.tensor_tensor(
                            out=lg[:, q * LQ + n2 * 500:q * LQ + (n2 + 1) * 500],
                            in0=psl[:, n2 * 512:n2 * 512 + 500],
                            in1=bout[:, q * LQ + n2 * 500:q * LQ + (n2 + 1) * 500],
                            op=mybir.AluOpType.add)
                    nc.vector.tensor_reduce(
                        out=mq[:, q:q + 1], in_=lg[:, q * LQ:(q + 1) * LQ],
                        axis=mybir.AxisListType.X, op=mybir.AluOpType.max)
                    nc.sync.dma_start(out=out_ap[:, t, q * LQ:(q + 1) * LQ],
                                      in_=lg[:, q * LQ:(q + 1) * LQ])

                if stage < 2:
                    hT, xT, c = hT_new, xT, c_new
                    continue
                # ---- local argmax over the shard
                m1 = sp.tile([B, 1], F32, tag="m1", bufs=2)
                nc.vector.tensor_reduce(out=m1[:], in_=mq[:],
                                        axis=mybir.AxisListType.X,
                                        op=mybir.AluOpType.max)
                gm8 = sp.tile([B, 8], F32, tag="gm8", bufs=2)
                nc.vector.tensor_copy(out=gm8[:], in_=m1[:].to_broadcast([B, 8]))
                idx8 = sp.tile([B, 8], U32, tag="idx8", bufs=2)
                nc.vector.max_index(idx8[:], gm8[:], lg[:])
                pack = sp.tile([B, 2], F32, tag="pack", bufs=2)
                nc.vector.tensor_copy(out=pack[:, 0:1], in_=m1[:])
                # global idx = local idx + rank * VL
                idxf = sp.tile([B, 1], F32, tag="idxf", bufs=2)
                nc.vector.tensor_copy(out=idxf[:], in_=idx8[:, 0:1])
                nc.vector.tensor_tensor(out=pack[:, 1:2], in0=idxf[:],
                                        in1=vbase[:], op=mybir.AluOpType.add)

                if stage < 3:
                    hT, xT, c = hT_new, xT, c_new
                    continue
                # ---- exchange (val, idx) with all cores
                cin = dp.tile([B, 2], F32, tag="cin", bufs=2)
                cout = dp.tile([NCORES, B, 2], F32, tag="cout", bufs=2)
                nc.sync.dma_start(out=cin[:], in_=pack[:])
                nc.gpsimd.collective_compute(
                    "AllGather", mybir.AluOpType.bypass,
                    replica_groups=[list(range(NCORES))],
                    ins=[cin[:].opt()], outs=[cout[:].opt()])
                allg = sp.tile([B, NCORES, 2], F32, tag="allg", bufs=2)
                nc.sync.dma_start(out=allg[:],
                                  in_=cout[:].rearrange("r b k -> b r k"))

                # ---- global winner: max value, lowest rank on ties
                vals8 = sp.tile([B, 8], F32, tag="vals8", bufs=2)
                idxs = sp.tile([B, 8], F32, tag="idxs8", bufs=2)
                nc.vector.tensor_copy(
                    out=vals8[:], in_=allg[:, :, 0:1].rearrange("b r k -> b (r k)"))
                nc.vector.tensor_copy(
                    out=idxs[:], in_=allg[:, :, 1:2].rearrange("b r k -> b (r k)"))
                wm8 = sp.tile([B, 8], F32, tag="wm8", bufs=2)
                nc.vector.max(wm8[:], vals8[:])
                wr8 = sp.tile([B, 8], U32, tag="wr8", bufs=2)
                nc.vector.max_index(wr8[:], wm8[:], vals8[:])
                wrf = sp.tile([B, 1], F32, tag="wrf", bufs=2)
                nc.vector.tensor_copy(out=wrf[:], in_=wr8[:, 0:1])
                sel = sp.tile([B, 8], F32, tag="sel", bufs=2)
                nc.vector.tensor_tensor(out=sel[:], in0=iota8[:],
                                        in1=wrf[:].to_broadcast([B, 8]),
                                        op=mybir.AluOpType.is_equal)
                nc.vector.tensor_tensor(out=sel[:], in0=sel[:], in1=idxs[:],
                                        op=mybir.AluOpType.mult)
                gidxf = sp.tile([B, 1], F32, tag="gidxf", bufs=2)
                nc.vector.tensor_reduce(out=gidxf[:], in_=sel[:],
                                        axis=mybir.AxisListType.X,
                                        op=mybir.AluOpType.add)
                gidx = sp.tile([B, 1], I32, tag="gidx", bufs=2)
                nc.vector.tensor_copy(out=gidx[:], in_=gidxf[:])

                if stage < 4:
                    hT, xT, c = hT_new, xT, c_new
                    continue
                # ---- feedback: x = emb[gidx]  (gather from full local copy)
                x_sb = sp.tile([B, E], F32, tag="x", bufs=2)
                nc.gpsimd.indirect_dma_start(
                    out=x_sb[:], out_offset=None,
                    in_=d_emb.ap()[:],
                    in_offset=bass.IndirectOffsetOnAxis(ap=gidx[:, :1], axis=0))
                xT_new = sp.tile([128, 4, B], F32, tag="xT", bufs=2)
                psx = pp.tile([128, 4 * B], F32, tag="g", bufs=2)
                for k in range(4):
                    nc.tensor.transpose(out=psx[:, k * B:(k + 1) * B],
                                        in_=x_sb[:, k * 128:(k + 1) * 128],
                                        identity=ident[0:B, 0:B])
                nc.scalar.activation(xT_new[:].rearrange("p k b -> p (k b)"),
                                     psx[:],
                                     mybir.ActivationFunctionType.Copy)

                hT, xT, c = hT_new, xT_new, c_new

    nc.compile()
    return nc


def _prep_inputs(enc_hs, last_enc_h, bos, emb, W_ih, W_hh, b_ih, b_hh,
                 W_out, b_out):
    emb = np.ascontiguousarray(np.asarray(emb, np.float32))
    wcat_t = np.concatenate([np.asarray(W_ih, np.float32).T,
                             np.asarray(W_hh, np.float32).T], axis=0)  # [1024, 2048]
    wcat = np.ascontiguousarray(wcat_t.reshape(8, 128, 2048).transpose(1, 0, 2))
    bgate = (np.asarray(b_ih, np.float32) + np.asarray(b_hh, np.float32))[None, :]
    h0t = np.ascontiguousarray(
        np.asarray(last_enc_h, np.float32).T.reshape(4, 128, B).transpose(1, 0, 2))
    x0 = np.broadcast_to(np.asarray(bos, np.float32)[None, :], (B, E))
    x0t = np.ascontiguousarray(x0.T.reshape(4, 128, B).transpose(1, 0, 2))
    ones = np.ones((1, B), np.float32)
    ident = np.eye(128, dtype=np.float32)
    iota8 = np.broadcast_to(np.arange(8, dtype=np.float32)[None, :], (B, 8)).copy()
    W_out = np.asarray(W_out, np.float32)
    b_out = np.asarray(b_out, np.float32)

    in_maps = []
    for r in range(NCORES):
        sh = slice(r * VL, (r + 1) * VL)
        wout_r = np.ascontiguousarray(
            W_out[sh].T.reshape(4, 128, VL).transpose(1, 0, 2))
        bout_r = np.broadcast_to(b_out[sh][None, :], (B, VL)).copy()
        in_maps.append({
            "wcat": wcat, "bgate": bgate, "wout": wout_r, "bout": bout_r,
            "emb": emb, "h0t": h0t, "x0t": x0t, "ones": ones, "ident": ident,
            "iota8": iota8,
            "myvbase": np.full((B, 1), r * VL, np.float32),
        })
    return in_maps


_CACHE = {}


def _get_nc(T: int, stage: int = 4):
    key = (T, stage)
    if key not in _CACHE:
        _CACHE[key] = build(T, stage)
    return _CACHE[key]


def run(T, trace=False, tmpdir=None, stage=4, **inputs):
    nc = _get_nc(T, stage)
    in_maps = _prep_inputs(**inputs)
    res = bass_utils.run_bass_kernel_spmd(
        nc, in_maps, core_ids=list(range(NCORES)), trace=trace, tmpdir=tmpdir)
    full = np.concatenate(
        [res.results[r]["logits"] for r in range(NCORES)], axis=2)  # [B, T, VL*8]
    dummy = np.zeros((B, T), np.float32)
    return (full, dummy, dummy), res


def kernel(**inputs):
    out, _ = run(64, **inputs)
    return out


# revision 7
# speedup vs baseline: 1.1217x; 1.1217x over previous
"""Greedy autoregressive LSTM decoder on 8 TRN2 NeuronCores.

Strategy: vocab-shard the hidden->vocab projection and the embedding-table
argmax across the 8 cores (4000 vocab rows each, W_out shard resident in
SBUF); replicate the small LSTM weights and redundantly compute the LSTM
cell on every core. Each step every core computes its local logits shard
and local (max, argmax); an 8-core AllGather of the 64 (val, idx) pairs
resolves the global argmax; each core then gathers the winning embedding
rows from its own full copy of the table in DRAM (indirect DMA) and feeds
them back. Logit shards stream to DRAM as they are produced and the host
concatenates along vocab.
"""

import sys

sys.path.insert(0, "/opt/trn_rl_repo")

import numpy as np

import concourse.bacc as bacc
import concourse.bass as bass
import concourse.tile as tile
from concourse import bass_utils, mybir

F32 = mybir.dt.float32
F32R = mybir.dt.float32r
BF16 = mybir.dt.bfloat16
I32 = mybir.dt.int32
U32 = mybir.dt.uint32

B = 64          # batch
E = 512         # embed
H = 512         # hidden
V = 32000       # vocab
NCORES = 8
VL = V // NCORES  # vocab shard per core
GQ = 1024       # gate-psum half width (i,f | g,o)
LQ = 1000       # logits quarter width


def build(T: int, stage: int = 4):
    nc = bacc.Bacc("TRN2", target_bir_lowering=False, debug=False,
                   num_devices=NCORES)

    # ---- kernel I/O ----
    d_wcat = nc.dram_tensor("wcat", [128, 8, 2048], F32, kind="ExternalInput")
    d_bgate = nc.dram_tensor("bgate", [1, 2048], F32, kind="ExternalInput")
    d_woutb = nc.dram_tensor("woutb", [128, 4, VL], BF16, kind="ExternalInput")
    d_woutl = nc.dram_tensor("woutl", [128, 4, VL], BF16, kind="ExternalInput")
    d_bout = nc.dram_tensor("bout", [B, VL], F32, kind="ExternalInput")
    d_emb = nc.dram_tensor("emb", [V, E], F32, kind="ExternalInput")
    d_h0t = nc.dram_tensor("h0t", [128, 4, B], F32, kind="ExternalInput")
    d_x0t = nc.dram_tensor("x0t", [128, 4, B], F32, kind="ExternalInput")
    d_ones = nc.dram_tensor("ones", [1, B], F32, kind="ExternalInput")
    d_ident = nc.dram_tensor("ident", [128, 128], F32, kind="ExternalInput")
    d_iota8 = nc.dram_tensor("iota8", [B, 8], F32, kind="ExternalInput")
    d_vbase = nc.dram_tensor("myvbase", [B, 1], F32, kind="ExternalInput")
    d_out = nc.dram_tensor("logits", [B, T, VL], F32, kind="ExternalOutput")
    out_ap = d_out.ap()

    with tile.TileContext(nc) as tc:
        with tc.tile_pool(name="w", bufs=1) as wp, \
             tc.tile_pool(name="s", bufs=2) as sp, \
             tc.tile_pool(name="ps", bufs=2, space="PSUM") as pp, \
             tc.tile_pool(name="dr", bufs=2, space="DRAM") as dp:

            # ---- preload weights/constants ----
            wcat = wp.tile([128, 8, 2048], F32)
            woutb = wp.tile([128, 4, VL], BF16)
            woutl = wp.tile([128, 4, VL], BF16)
            bgate = wp.tile([1, 2048], F32)
            bout = wp.tile([B, VL], F32)
            ones = wp.tile([1, B], F32)
            ident = wp.tile([128, 128], F32)
            iota8 = wp.tile([B, 8], F32)
            vbase = wp.tile([B, 1], F32)
            for dst, src in ((wcat, d_wcat), (woutb, d_woutb), (woutl, d_woutl),
                             (bgate, d_bgate), (bout, d_bout), (ones, d_ones),
                             (ident, d_ident), (iota8, d_iota8), (vbase, d_vbase)):
                nc.sync.dma_start(out=dst[:], in_=src.ap()[:])
            # float32r copies for the gate-bias matmuls (4x faster than fp32)
            ones_r = wp.tile([1, B], F32R)
            bgate_r = wp.tile([1, 2048], F32R)
            nc.vector.tensor_copy(out=ones_r[:], in_=ones[:])
            nc.vector.tensor_copy(out=bgate_r[:], in_=bgate[:])

            # ---- initial state ----
            hT = sp.tile([128, 4, B], F32, tag="hT", bufs=2)
            xT = sp.tile([128, 4, B], F32, tag="xT", bufs=2)
            c = sp.tile([B, H], F32, tag="c", bufs=2)
            nc.sync.dma_start(out=hT[:], in_=d_h0t.ap()[:])
            nc.sync.dma_start(out=xT[:], in_=d_x0t.ap()[:])
            nc.vector.memset(c[:], 0.0)

            for t in range(T):
                # ---- LSTM gates:  z = [x, h],  gates = z @ [Wih.T; Whh.T] + b
                # two halves: cols 0..1023 = (i, f), 1024..2047 = (g, o)
                gact = []  # sigmoid(i), sigmoid(f), tanh(g), sigmoid(o)
                for half in range(2):
                    psg = pp.tile([64, GQ], F32, tag="g", bufs=2)
                    for k in range(8):
                        zk = xT[:, k, :] if k < 4 else hT[:, k - 4, :]
                        for n2 in range(2):
                            nc.tensor.matmul(
                                out=psg[:, n2 * 512:(n2 + 1) * 512],
                                lhsT=zk,
                                rhs=wcat[:, k, half * GQ + n2 * 512:
                                         half * GQ + (n2 + 1) * 512],
                                start=(k == 0), stop=False)
                    for n2 in range(2):
                        nc.tensor.matmul(
                            out=psg[:, n2 * 512:(n2 + 1) * 512],
                            lhsT=ones_r[:, :],
                            rhs=bgate_r[:, half * GQ + n2 * 512:
                                        half * GQ + (n2 + 1) * 512],
                            start=False, stop=True)
                    for n2 in range(2):
                        act = sp.tile([B, 512], F32, tag=f"ga{half}{n2}", bufs=1)
                        fn = (mybir.ActivationFunctionType.Tanh
                              if (half, n2) == (1, 0)
                              else mybir.ActivationFunctionType.Sigmoid)
                        nc.scalar.activation(act[:], psg[:, n2 * 512:(n2 + 1) * 512], fn)
                        gact.append(act)

                si, sf, tg, so = gact
                # c' = sf * c + si * tg ;  h = so * tanh(c')
                t1 = sp.tile([B, H], F32, tag="t1", bufs=1)
                nc.vector.tensor_tensor(out=t1[:], in0=si[:], in1=tg[:],
                                        op=mybir.AluOpType.mult)
                c_new = sp.tile([B, H], F32, tag="c", bufs=2)
                nc.vector.tensor_tensor(out=c_new[:], in0=sf[:], in1=c[:],
                                        op=mybir.AluOpType.mult)
                nc.vector.tensor_tensor(out=c_new[:], in0=c_new[:], in1=t1[:],
                                        op=mybir.AluOpType.add)
                tc_ = sp.tile([B, H], F32, tag="tc", bufs=1)
                nc.scalar.activation(tc_[:], c_new[:],
                                     mybir.ActivationFunctionType.Tanh)
                h_new = sp.tile([B, H], F32, tag="h", bufs=1)
                nc.vector.tensor_tensor(out=h_new[:], in0=so[:], in1=tc_[:],
                                        op=mybir.AluOpType.mult)

                # ---- transpose h -> hT tiles for next matmuls
                hT_new = sp.tile([128, 4, B], F32, tag="hT", bufs=2)
                pst = pp.tile([128, 4 * B], F32, tag="g", bufs=2)
                for k in range(4):
                    nc.tensor.transpose(out=pst[:, k * B:(k + 1) * B],
                                        in_=h_new[:, k * 128:(k + 1) * 128],
                                        identity=ident[0:B, 0:B])
                nc.scalar.activation(hT_new[:].rearrange("p k b -> p (k b)"),
                                     pst[:],
                                     mybir.ActivationFunctionType.Copy)
                # split hT into bf16 head + bf16 residual for 3-term matmul
                hbT = sp.tile([128, 4, B], BF16, tag="hbT", bufs=2)
                hrT = sp.tile([128, 4, B], F32, tag="hrT", bufs=2)
                hlT = sp.tile([128, 4, B], BF16, tag="hlT", bufs=2)
                nc.vector.tensor_copy(out=hbT[:], in_=hT_new[:])
                nc.vector.tensor_tensor(out=hrT[:], in0=hT_new[:], in1=hbT[:],
                                        op=mybir.AluOpType.subtract)
                nc.vector.tensor_copy(out=hlT[:], in_=hrT[:])

                # ---- logits shard:  lg = h @ Wout_loc.T + bout_loc   [B, VL]
                lg = sp.tile([B, VL], F32, tag="lg", bufs=1)
                mq = sp.tile([B, 4], F32, tag="mq", bufs=2)
                for q in range(4):
                    psl = pp.tile([64, GQ], F32, tag="l", bufs=2)
                    terms = [(hbT, woutb), (hbT, woutl), (hlT, woutb)]
                    for k in range(4):
                        for ti, (lh, rh) in enumerate(terms):
                            first = (k == 0 and ti == 0)
                            last = (k == 3 and ti == len(terms) - 1)
                            for n2 in range(2):
                                nc.tensor.matmul(
                                    out=psl[:, n2 * 512:n2 * 512 + 500],
                                    lhsT=lh[:, k, :],
                                    rhs=rh[:, k, q * LQ + n2 * 500:
                                           q * LQ + (n2 + 1) * 500],
                                    start=first, stop=last)
                    # bias add + copy to SBUF, then per-quarter max
                    for n2 in range(2):
                        nc.vector.tensor_tensor(
                            out=lg[:, q * LQ + n2 * 500:q * LQ + (n2 + 1) * 500],
                            in0=psl[:, n2 * 512:n2 * 512 + 500],
                            in1=bout[:, q * LQ + n2 * 500:q * LQ + (n2 + 1) * 500],
                            op=mybir.AluOpType.add)
                    nc.vector.tensor_reduce(
                        out=mq[:, q:q + 1], in_=lg[:, q * LQ:(q + 1) * LQ],
                        axis=mybir.AxisListType.X, op=mybir.AluOpType.max)
                    nc.sync.dma_start(out=out_ap[:, t, q * LQ:(q + 1) * LQ],
                                      in_=lg[:, q * LQ:(q + 1) * LQ])

                if stage < 2:
                    hT, xT, c = hT_new, xT, c_new
                    continue
                # ---- local argmax over the shard
                m1 = sp.tile([B, 1], F32, tag="m1", bufs=2)
                nc.vector.tensor_reduce(out=m1[:], in_=mq[:],
                                        axis=mybir.AxisListType.X,
                                        op=mybir.AluOpType.max)
                gm8 = sp.tile([B, 8], F32, tag="gm8", bufs=2)
                nc.vector.tensor_copy(out=gm8[:], in_=m1[:].to_broadcast([B, 8]))
                idx8 = sp.tile([B, 8], U32, tag="idx8", bufs=2)
                nc.vector.max_index(idx8[:], gm8[:], lg[:])
                pack = sp.tile([B, 2], F32, tag="pack", bufs=2)
                nc.vector.tensor_copy(out=pack[:, 0:1], in_=m1[:])
                # global idx = local idx + rank * VL
                idxf = sp.tile([B, 1], F32, tag="idxf", bufs=2)
                nc.vector.tensor_copy(out=idxf[:], in_=idx8[:, 0:1])
                nc.vector.tensor_tensor(out=pack[:, 1:2], in0=idxf[:],
                                        in1=vbase[:], op=mybir.AluOpType.add)

                if stage < 3:
                    hT, xT, c = hT_new, xT, c_new
                    continue
                # ---- exchange (val, idx) with all cores
                cin = dp.tile([B, 2], F32, tag="cin", bufs=2)
                cout = dp.tile([NCORES, B, 2], F32, tag="cout", bufs=2)
                nc.sync.dma_start(out=cin[:], in_=pack[:])
                nc.gpsimd.collective_compute(
                    "AllGather", mybir.AluOpType.bypass,
                    replica_groups=[list(range(NCORES))],
                    ins=[cin[:].opt()], outs=[cout[:].opt()])
                allg = sp.tile([B, NCORES, 2], F32, tag="allg", bufs=2)
                nc.sync.dma_start(out=allg[:],
                                  in_=cout[:].rearrange("r b k -> b r k"))

                # ---- global winner: max value, lowest rank on ties
                vals8 = sp.tile([B, 8], F32, tag="vals8", bufs=2)
                idxs = sp.tile([B, 8], F32, tag="idxs8", bufs=2)
                nc.vector.tensor_copy(
                    out=vals8[:], in_=allg[:, :, 0:1].rearrange("b r k -> b (r k)"))
                nc.vector.tensor_copy(
                    out=idxs[:], in_=allg[:, :, 1:2].rearrange("b r k -> b (r k)"))
                wm8 = sp.tile([B, 8], F32, tag="wm8", bufs=2)
                nc.vector.max(wm8[:], vals8[:])
                wr8 = sp.tile([B, 8], U32, tag="wr8", bufs=2)
                nc.vector.max_index(wr8[:], wm8[:], vals8[:])
                wrf = sp.tile([B, 1], F32, tag="wrf", bufs=2)
                nc.vector.tensor_copy(out=wrf[:], in_=wr8[:, 0:1])
                sel = sp.tile([B, 8], F32, tag="sel", bufs=2)
                nc.vector.tensor_tensor(out=sel[:], in0=iota8[:],
                                        in1=wrf[:].to_broadcast([B, 8]),
                                        op=mybir.AluOpType.is_equal)
                nc.vector.tensor_tensor(out=sel[:], in0=sel[:], in1=idxs[:],
                                        op=mybir.AluOpType.mult)
                gidxf = sp.tile([B, 1], F32, tag="gidxf", bufs=2)
                nc.vector.tensor_reduce(out=gidxf[:], in_=sel[:],
                                        axis=mybir.AxisListType.X,
                                        op=mybir.AluOpType.add)
                gidx = sp.tile([B, 1], I32, tag="gidx", bufs=2)
                nc.vector.tensor_copy(out=gidx[:], in_=gidxf[:])

                if stage < 4:
                    hT, xT, c = hT_new, xT, c_new
                    continue
                # ---- feedback: x = emb[gidx]  (gather from full local copy)
                x_sb = sp.tile([B, E], F32, tag="x", bufs=2)
                nc.gpsimd.indirect_dma_start(
                    out=x_sb[:], out_offset=None,
                    in_=d_emb.ap()[:],
                    in_offset=bass.IndirectOffsetOnAxis(ap=gidx[:, :1], axis=0))
                xT_new = sp.tile([128, 4, B], F32, tag="xT", bufs=2)
                psx = pp.tile([128, 4 * B], F32, tag="g", bufs=2)
                for k in range(4):
                    nc.tensor.transpose(out=psx[:, k * B:(k + 1) * B],
                                        in_=x_sb[:, k * 128:(k + 1) * 128],
                                        identity=ident[0:B, 0:B])
                nc.scalar.activation(xT_new[:].rearrange("p k b -> p (k b)"),
                                     psx[:],
                                     mybir.ActivationFunctionType.Copy)

                hT, xT, c = hT_new, xT_new, c_new

    nc.compile()
    return nc


def _prep_inputs(enc_hs, last_enc_h, bos, emb, W_ih, W_hh, b_ih, b_hh,
                 W_out, b_out):
    emb = np.ascontiguousarray(np.asarray(emb, np.float32))
    wcat_t = np.concatenate([np.asarray(W_ih, np.float32).T,
                             np.asarray(W_hh, np.float32).T], axis=0)  # [1024, 2048]
    wcat = np.ascontiguousarray(wcat_t.reshape(8, 128, 2048).transpose(1, 0, 2))
    bgate = (np.asarray(b_ih, np.float32) + np.asarray(b_hh, np.float32))[None, :]
    h0t = np.ascontiguousarray(
        np.asarray(last_enc_h, np.float32).T.reshape(4, 128, B).transpose(1, 0, 2))
    x0 = np.broadcast_to(np.asarray(bos, np.float32)[None, :], (B, E))
    x0t = np.ascontiguousarray(x0.T.reshape(4, 128, B).transpose(1, 0, 2))
    ones = np.ones((1, B), np.float32)
    ident = np.eye(128, dtype=np.float32)
    iota8 = np.broadcast_to(np.arange(8, dtype=np.float32)[None, :], (B, 8)).copy()
    W_out = np.asarray(W_out, np.float32)
    b_out = np.asarray(b_out, np.float32)

    in_maps = []
    import ml_dtypes
    for r in range(NCORES):
        sh = slice(r * VL, (r + 1) * VL)
        wout_r = np.ascontiguousarray(
            W_out[sh].T.reshape(4, 128, VL).transpose(1, 0, 2))
        woutb_r = wout_r.astype(ml_dtypes.bfloat16)
        woutl_r = (wout_r - woutb_r.astype(np.float32)).astype(ml_dtypes.bfloat16)
        bout_r = np.broadcast_to(b_out[sh][None, :], (B, VL)).copy()
        in_maps.append({
            "wcat": wcat, "bgate": bgate, "woutb": woutb_r, "woutl": woutl_r,
            "bout": bout_r,
            "emb": emb, "h0t": h0t, "x0t": x0t, "ones": ones, "ident": ident,
            "iota8": iota8,
            "myvbase": np.full((B, 1), r * VL, np.float32),
        })
    return in_maps


_CACHE = {}


def _get_nc(T: int, stage: int = 4):
    key = (T, stage)
    if key not in _CACHE:
        _CACHE[key] = build(T, stage)
    return _CACHE[key]


def run(T, trace=False, tmpdir=None, stage=4, **inputs):
    nc = _get_nc(T, stage)
    in_maps = _prep_inputs(**inputs)
    res = bass_utils.run_bass_kernel_spmd(
        nc, in_maps, core_ids=list(range(NCORES)), trace=trace, tmpdir=tmpdir)
    full = np.concatenate(
        [res.results[r]["logits"] for r in range(NCORES)], axis=2)  # [B, T, VL*8]
    dummy = np.zeros((B, T), np.float32)
    return (full, dummy, dummy), res


def kernel(**inputs):
    out, _ = run(64, **inputs)
    return out


# revision 8
# speedup vs baseline: 1.1394x; 1.0157x over previous
"""Greedy autoregressive LSTM decoder on 8 TRN2 NeuronCores.

Strategy: vocab-shard the hidden->vocab projection and the embedding-table
argmax across the 8 cores (4000 vocab rows each, W_out shard resident in
SBUF); replicate the small LSTM weights and redundantly compute the LSTM
cell on every core. Each step every core computes its local logits shard
and local (max, argmax); an 8-core AllGather of the 64 (val, idx) pairs
resolves the global argmax; each core then gathers the winning embedding
rows from its own full copy of the table in DRAM (indirect DMA) and feeds
them back. Logit shards stream to DRAM as they are produced and the host
concatenates along vocab.
"""

import sys

sys.path.insert(0, "/opt/trn_rl_repo")

import numpy as np

import concourse.bacc as bacc
import concourse.bass as bass
import concourse.tile as tile
from concourse import bass_utils, mybir

F32 = mybir.dt.float32
F32R = mybir.dt.float32r
BF16 = mybir.dt.bfloat16
I32 = mybir.dt.int32
U32 = mybir.dt.uint32

B = 64          # batch
E = 512         # embed
H = 512         # hidden
V = 32000       # vocab
NCORES = 8
VL = V // NCORES  # vocab shard per core
GQ = 1024       # gate-psum half width (i,f | g,o)
LQ = 1000       # logits quarter width


def build(T: int, stage: int = 4):
    nc = bacc.Bacc("TRN2", target_bir_lowering=False, debug=False,
                   num_devices=NCORES)

    # ---- kernel I/O ----
    d_wcat = nc.dram_tensor("wcat", [128, 8, 2048], F32, kind="ExternalInput")
    d_bgate = nc.dram_tensor("bgate", [1, 2048], F32, kind="ExternalInput")
    d_woutb = nc.dram_tensor("woutb", [128, 4, VL], BF16, kind="ExternalInput")
    d_woutl = nc.dram_tensor("woutl", [128, 4, VL], BF16, kind="ExternalInput")
    d_bout = nc.dram_tensor("bout", [B, VL], F32, kind="ExternalInput")
    d_emb = nc.dram_tensor("emb", [V, E], F32, kind="ExternalInput")
    d_h0t = nc.dram_tensor("h0t", [128, 4, B], F32, kind="ExternalInput")
    d_x0t = nc.dram_tensor("x0t", [128, 4, B], F32, kind="ExternalInput")
    d_ones = nc.dram_tensor("ones", [1, B], F32, kind="ExternalInput")
    d_ident = nc.dram_tensor("ident", [128, 128], F32, kind="ExternalInput")
    d_iota8 = nc.dram_tensor("iota8", [B, 8], F32, kind="ExternalInput")
    d_vbase = nc.dram_tensor("myvbase", [B, 1], F32, kind="ExternalInput")
    d_out = nc.dram_tensor("logits", [B, T, VL], F32, kind="ExternalOutput")
    out_ap = d_out.ap()

    with tile.TileContext(nc) as tc:
        with tc.tile_pool(name="w", bufs=1) as wp, \
             tc.tile_pool(name="s", bufs=2) as sp, \
             tc.tile_pool(name="ps", bufs=2, space="PSUM") as pp, \
             tc.tile_pool(name="dr", bufs=2, space="DRAM") as dp:

            # ---- preload weights/constants ----
            wcat = wp.tile([128, 8, 2048], F32)
            woutb = wp.tile([128, 4, VL], BF16)
            woutl = wp.tile([128, 4, VL], BF16)
            bgate = wp.tile([1, 2048], F32)
            bout = wp.tile([B, VL], F32)
            ones = wp.tile([1, B], F32)
            ident = wp.tile([128, 128], F32)
            iota8 = wp.tile([B, 8], F32)
            vbase = wp.tile([B, 1], F32)
            for dst, src in ((wcat, d_wcat), (woutb, d_woutb), (woutl, d_woutl),
                             (bgate, d_bgate), (bout, d_bout), (ones, d_ones),
                             (ident, d_ident), (iota8, d_iota8), (vbase, d_vbase)):
                nc.sync.dma_start(out=dst[:], in_=src.ap()[:])
            # float32r copies for the gate-bias matmuls (4x faster than fp32)
            ones_r = wp.tile([1, B], F32R)
            bgate_r = wp.tile([1, 2048], F32R)
            nc.vector.tensor_copy(out=ones_r[:], in_=ones[:])
            nc.vector.tensor_copy(out=bgate_r[:], in_=bgate[:])

            # ---- initial state ----
            hT = sp.tile([128, 4, B], F32, tag="hT", bufs=2)
            xT = sp.tile([128, 4, B], F32, tag="xT", bufs=2)
            c = sp.tile([B, H], F32, tag="c", bufs=2)
            nc.sync.dma_start(out=hT[:], in_=d_h0t.ap()[:])
            nc.sync.dma_start(out=xT[:], in_=d_x0t.ap()[:])
            nc.vector.memset(c[:], 0.0)

            def heartbeat(ap):
                w = ap.bitcast(BF16)
                p = min(w.shape[0], 128)
                f = min(w.free_size() // (1 if len(w.shape) == 1 else 1), 128)
                nc.tensor.ldweights(weights=w[0:p, 0:min(f, 128)])

            for t in range(T):
                # ---- LSTM gates:  z = [x, h],  gates = z @ [Wih.T; Whh.T] + b
                # two halves: cols 0..1023 = (i, f), 1024..2047 = (g, o)
                gact = []  # sigmoid(i), sigmoid(f), tanh(g), sigmoid(o)
                for half in range(2):
                    psg = pp.tile([64, GQ], F32, tag="g", bufs=2)
                    for k in range(8):
                        zk = xT[:, k, :] if k < 4 else hT[:, k - 4, :]
                        for n2 in range(2):
                            nc.tensor.matmul(
                                out=psg[:, n2 * 512:(n2 + 1) * 512],
                                lhsT=zk,
                                rhs=wcat[:, k, half * GQ + n2 * 512:
                                         half * GQ + (n2 + 1) * 512],
                                start=(k == 0), stop=False)
                    for n2 in range(2):
                        nc.tensor.matmul(
                            out=psg[:, n2 * 512:(n2 + 1) * 512],
                            lhsT=ones_r[:, :],
                            rhs=bgate_r[:, half * GQ + n2 * 512:
                                        half * GQ + (n2 + 1) * 512],
                            start=False, stop=True)
                    for n2 in range(2):
                        act = sp.tile([B, 512], F32, tag=f"ga{half}{n2}", bufs=1)
                        fn = (mybir.ActivationFunctionType.Tanh
                              if (half, n2) == (1, 0)
                              else mybir.ActivationFunctionType.Sigmoid)
                        nc.scalar.activation(act[:], psg[:, n2 * 512:(n2 + 1) * 512], fn)
                        gact.append(act)

                si, sf, tg, so = gact
                # c' = sf * c + si * tg ;  h = so * tanh(c')
                t1 = sp.tile([B, H], F32, tag="t1", bufs=1)
                nc.vector.tensor_tensor(out=t1[:], in0=si[:], in1=tg[:],
                                        op=mybir.AluOpType.mult)
                c_new = sp.tile([B, H], F32, tag="c", bufs=2)
                nc.vector.tensor_tensor(out=c_new[:], in0=sf[:], in1=c[:],
                                        op=mybir.AluOpType.mult)
                nc.vector.tensor_tensor(out=c_new[:], in0=c_new[:], in1=t1[:],
                                        op=mybir.AluOpType.add)
                tc_ = sp.tile([B, H], F32, tag="tc", bufs=1)
                nc.scalar.activation(tc_[:], c_new[:],
                                     mybir.ActivationFunctionType.Tanh)
                h_new = sp.tile([B, H], F32, tag="h", bufs=1)
                nc.vector.tensor_tensor(out=h_new[:], in0=so[:], in1=tc_[:],
                                        op=mybir.AluOpType.mult)

                # ---- transpose h -> hT tiles for next matmuls
                hT_new = sp.tile([128, 4, B], F32, tag="hT", bufs=2)
                pst = pp.tile([128, 4 * B], F32, tag="g", bufs=2)
                for k in range(4):
                    nc.tensor.transpose(out=pst[:, k * B:(k + 1) * B],
                                        in_=h_new[:, k * 128:(k + 1) * 128],
                                        identity=ident[0:B, 0:B])
                nc.scalar.activation(hT_new[:].rearrange("p k b -> p (k b)"),
                                     pst[:],
                                     mybir.ActivationFunctionType.Copy)
                # split hT into bf16 head + bf16 residual for 3-term matmul
                hbT = sp.tile([128, 4, B], BF16, tag="hbT", bufs=2)
                hrT = sp.tile([128, 4, B], F32, tag="hrT", bufs=2)
                hlT = sp.tile([128, 4, B], BF16, tag="hlT", bufs=2)
                nc.vector.tensor_copy(out=hbT[:], in_=hT_new[:])
                nc.vector.tensor_tensor(out=hrT[:], in0=hT_new[:], in1=hbT[:],
                                        op=mybir.AluOpType.subtract)
                nc.vector.tensor_copy(out=hlT[:], in_=hrT[:])

                # ---- logits shard:  lg = h @ Wout_loc.T + bout_loc   [B, VL]
                lg = sp.tile([B, VL], F32, tag="lg", bufs=1)
                mq = sp.tile([B, 4], F32, tag="mq", bufs=2)
                for q in range(4):
                    psl = pp.tile([64, GQ], F32, tag="l", bufs=2)
                    terms = [(hbT, woutb), (hbT, woutl), (hlT, woutb)]
                    for k in range(4):
                        for ti, (lh, rh) in enumerate(terms):
                            first = (k == 0 and ti == 0)
                            last = (k == 3 and ti == len(terms) - 1)
                            for n2 in range(2):
                                nc.tensor.matmul(
                                    out=psl[:, n2 * 512:n2 * 512 + 500],
                                    lhsT=lh[:, k, :],
                                    rhs=rh[:, k, q * LQ + n2 * 500:
                                           q * LQ + (n2 + 1) * 500],
                                    start=first, stop=last)
                    # bias add + copy to SBUF, then per-quarter max
                    for n2 in range(2):
                        nc.vector.tensor_tensor(
                            out=lg[:, q * LQ + n2 * 500:q * LQ + (n2 + 1) * 500],
                            in0=psl[:, n2 * 512:n2 * 512 + 500],
                            in1=bout[:, q * LQ + n2 * 500:q * LQ + (n2 + 1) * 500],
                            op=mybir.AluOpType.add)
                    nc.vector.tensor_reduce(
                        out=mq[:, q:q + 1], in_=lg[:, q * LQ:(q + 1) * LQ],
                        axis=mybir.AxisListType.X, op=mybir.AluOpType.max)
                    heartbeat(lg[:, q * LQ:q * LQ + 64])
                    nc.sync.dma_start(out=out_ap[:, t, q * LQ:(q + 1) * LQ],
                                      in_=lg[:, q * LQ:(q + 1) * LQ])

                if stage < 2:
                    hT, xT, c = hT_new, xT, c_new
                    continue
                # ---- local argmax over the shard
                m1 = sp.tile([B, 1], F32, tag="m1", bufs=2)
                nc.vector.tensor_reduce(out=m1[:], in_=mq[:],
                                        axis=mybir.AxisListType.X,
                                        op=mybir.AluOpType.max)
                gm8 = sp.tile([B, 8], F32, tag="gm8", bufs=2)
                nc.vector.tensor_copy(out=gm8[:], in_=m1[:].to_broadcast([B, 8]))
                idx8 = sp.tile([B, 8], U32, tag="idx8", bufs=2)
                nc.vector.max_index(idx8[:], gm8[:], lg[:])
                heartbeat(idx8[:])
                pack = sp.tile([B, 2], F32, tag="pack", bufs=2)
                nc.vector.tensor_copy(out=pack[:, 0:1], in_=m1[:])
                # global idx = local idx + rank * VL
                idxf = sp.tile([B, 1], F32, tag="idxf", bufs=2)
                nc.vector.tensor_copy(out=idxf[:], in_=idx8[:, 0:1])
                nc.vector.tensor_tensor(out=pack[:, 1:2], in0=idxf[:],
                                        in1=vbase[:], op=mybir.AluOpType.add)
                heartbeat(pack[:])

                if stage < 3:
                    hT, xT, c = hT_new, xT, c_new
                    continue
                # ---- exchange (val, idx) with all cores
                cin = dp.tile([B, 2], F32, tag="cin", bufs=2)
                cout = dp.tile([NCORES, B, 2], F32, tag="cout", bufs=2)
                nc.sync.dma_start(out=cin[:], in_=pack[:])
                nc.gpsimd.collective_compute(
                    "AllGather", mybir.AluOpType.bypass,
                    replica_groups=[list(range(NCORES))],
                    ins=[cin[:].opt()], outs=[cout[:].opt()])
                allg = sp.tile([B, NCORES, 2], F32, tag="allg", bufs=2)
                nc.sync.dma_start(out=allg[:],
                                  in_=cout[:].rearrange("r b k -> b r k"))

                # ---- global winner: max value, lowest rank on ties
                vals8 = sp.tile([B, 8], F32, tag="vals8", bufs=2)
                idxs = sp.tile([B, 8], F32, tag="idxs8", bufs=2)
                nc.vector.tensor_copy(
                    out=vals8[:], in_=allg[:, :, 0:1].rearrange("b r k -> b (r k)"))
                heartbeat(vals8[:])
                nc.vector.tensor_copy(
                    out=idxs[:], in_=allg[:, :, 1:2].rearrange("b r k -> b (r k)"))
                wm8 = sp.tile([B, 8], F32, tag="wm8", bufs=2)
                nc.vector.max(wm8[:], vals8[:])
                wr8 = sp.tile([B, 8], U32, tag="wr8", bufs=2)
                nc.vector.max_index(wr8[:], wm8[:], vals8[:])
                wrf = sp.tile([B, 1], F32, tag="wrf", bufs=2)
                nc.vector.tensor_copy(out=wrf[:], in_=wr8[:, 0:1])
                sel = sp.tile([B, 8], F32, tag="sel", bufs=2)
                nc.vector.tensor_tensor(out=sel[:], in0=iota8[:],
                                        in1=wrf[:].to_broadcast([B, 8]),
                                        op=mybir.AluOpType.is_equal)
                nc.vector.tensor_tensor(out=sel[:], in0=sel[:], in1=idxs[:],
                                        op=mybir.AluOpType.mult)
                gidxf = sp.tile([B, 1], F32, tag="gidxf", bufs=2)
                nc.vector.tensor_reduce(out=gidxf[:], in_=sel[:],
                                        axis=mybir.AxisListType.X,
                                        op=mybir.AluOpType.add)
                gidx = sp.tile([B, 1], I32, tag="gidx", bufs=2)
                nc.vector.tensor_copy(out=gidx[:], in_=gidxf[:])
                heartbeat(gidx[:])

                if stage < 4:
                    hT, xT, c = hT_new, xT, c_new
                    continue
                # ---- feedback: x = emb[gidx]  (gather from full local copy)
                x_sb = sp.tile([B, E], F32, tag="x", bufs=2)
                nc.gpsimd.indirect_dma_start(
                    out=x_sb[:], out_offset=None,
                    in_=d_emb.ap()[:],
                    in_offset=bass.IndirectOffsetOnAxis(ap=gidx[:, :1], axis=0))
                heartbeat(x_sb[:, 0:64])
                xT_new = sp.tile([128, 4, B], F32, tag="xT", bufs=2)
                psx = pp.tile([128, 4 * B], F32, tag="g", bufs=2)
                for k in range(4):
                    nc.tensor.transpose(out=psx[:, k * B:(k + 1) * B],
                                        in_=x_sb[:, k * 128:(k + 1) * 128],
                                        identity=ident[0:B, 0:B])
                nc.scalar.activation(xT_new[:].rearrange("p k b -> p (k b)"),
                                     psx[:],
                                     mybir.ActivationFunctionType.Copy)

                hT, xT, c = hT_new, xT_new, c_new

    nc.compile()
    return nc


def _prep_inputs(enc_hs, last_enc_h, bos, emb, W_ih, W_hh, b_ih, b_hh,
                 W_out, b_out):
    emb = np.ascontiguousarray(np.asarray(emb, np.float32))
    wcat_t = np.concatenate([np.asarray(W_ih, np.float32).T,
                             np.asarray(W_hh, np.float32).T], axis=0)  # [1024, 2048]
    wcat = np.ascontiguousarray(wcat_t.reshape(8, 128, 2048).transpose(1, 0, 2))
    bgate = (np.asarray(b_ih, np.float32) + np.asarray(b_hh, np.float32))[None, :]
    h0t = np.ascontiguousarray(
        np.asarray(last_enc_h, np.float32).T.reshape(4, 128, B).transpose(1, 0, 2))
    x0 = np.broadcast_to(np.asarray(bos, np.float32)[None, :], (B, E))
    x0t = np.ascontiguousarray(x0.T.reshape(4, 128, B).transpose(1, 0, 2))
    ones = np.ones((1, B), np.float32)
    ident = np.eye(128, dtype=np.float32)
    iota8 = np.broadcast_to(np.arange(8, dtype=np.float32)[None, :], (B, 8)).copy()
    W_out = np.asarray(W_out, np.float32)
    b_out = np.asarray(b_out, np.float32)

    in_maps = []
    import ml_dtypes
    for r in range(NCORES):
        sh = slice(r * VL, (r + 1) * VL)
        wout_r = np.ascontiguousarray(
            W_out[sh].T.reshape(4, 128, VL).transpose(1, 0, 2))
        woutb_r = wout_r.astype(ml_dtypes.bfloat16)
        woutl_r = (wout_r - woutb_r.astype(np.float32)).astype(ml_dtypes.bfloat16)
        bout_r = np.broadcast_to(b_out[sh][None, :], (B, VL)).copy()
        in_maps.append({
            "wcat": wcat, "bgate": bgate, "woutb": woutb_r, "woutl": woutl_r,
            "bout": bout_r,
            "emb": emb, "h0t": h0t, "x0t": x0t, "ones": ones, "ident": ident,
            "iota8": iota8,
            "myvbase": np.full((B, 1), r * VL, np.float32),
        })
    return in_maps


_CACHE = {}


def _get_nc(T: int, stage: int = 4):
    key = (T, stage)
    if key not in _CACHE:
        _CACHE[key] = build(T, stage)
    return _CACHE[key]


def run(T, trace=False, tmpdir=None, stage=4, **inputs):
    nc = _get_nc(T, stage)
    in_maps = _prep_inputs(**inputs)
    res = bass_utils.run_bass_kernel_spmd(
        nc, in_maps, core_ids=list(range(NCORES)), trace=trace, tmpdir=tmpdir)
    full = np.concatenate(
        [res.results[r]["logits"] for r in range(NCORES)], axis=2)  # [B, T, VL*8]
    dummy = np.zeros((B, T), np.float32)
    return (full, dummy, dummy), res


def kernel(**inputs):
    out, _ = run(64, **inputs)
    return out


# revision 9
# speedup vs baseline: 1.3875x; 1.2177x over previous
"""Greedy autoregressive LSTM decoder on 8 TRN2 NeuronCores.

Strategy: vocab-shard the hidden->vocab projection and the embedding-table
argmax across the 8 cores (4000 vocab rows each, W_out shard resident in
SBUF); replicate the small LSTM weights and redundantly compute the LSTM
cell on every core. Each step every core computes its local logits shard
and local (max, argmax); an 8-core AllGather of the 64 (val, idx) pairs
resolves the global argmax; each core then gathers the winning embedding
rows from its own full copy of the table in DRAM (indirect DMA) and feeds
them back. Logit shards stream to DRAM as they are produced and the host
concatenates along vocab.
"""

import sys

sys.path.insert(0, "/opt/trn_rl_repo")

import numpy as np

import concourse.bacc as bacc
import concourse.bass as bass
import concourse.tile as tile
from concourse import bass_utils, mybir

F32 = mybir.dt.float32
F32R = mybir.dt.float32r
BF16 = mybir.dt.bfloat16
I32 = mybir.dt.int32
U32 = mybir.dt.uint32

B = 64          # batch
E = 512         # embed
H = 512         # hidden
V = 32000       # vocab
NCORES = 8
VL = V // NCORES  # vocab shard per core
GQ = 1024       # gate-psum half width (i,f | g,o)
LQ = 1000       # logits quarter width


def build(T: int, stage: int = 4):
    nc = bacc.Bacc("TRN2", target_bir_lowering=False, debug=False,
                   num_devices=NCORES)

    # ---- kernel I/O ----
    d_wcat = nc.dram_tensor("wcat", [128, 8, 2048], F32, kind="ExternalInput")
    d_bgate = nc.dram_tensor("bgate", [1, 2048], F32, kind="ExternalInput")
    d_woutb = nc.dram_tensor("woutb", [128, 4, VL], BF16, kind="ExternalInput")
    d_woutl = nc.dram_tensor("woutl", [128, 4, VL], BF16, kind="ExternalInput")
    d_bout = nc.dram_tensor("bout", [B, VL], F32, kind="ExternalInput")
    d_emb = nc.dram_tensor("emb", [V, E], F32, kind="ExternalInput")
    d_h0t = nc.dram_tensor("h0t", [128, 4, B], F32, kind="ExternalInput")
    d_x0t = nc.dram_tensor("x0t", [128, 4, B], F32, kind="ExternalInput")
    d_ones = nc.dram_tensor("ones", [1, B], F32, kind="ExternalInput")
    d_ident = nc.dram_tensor("ident", [128, 128], F32, kind="ExternalInput")
    d_iota8 = nc.dram_tensor("iota8", [B, 8], F32, kind="ExternalInput")
    d_vbase = nc.dram_tensor("myvbase", [B, 1], F32, kind="ExternalInput")
    d_out = nc.dram_tensor("logits", [B, T, VL], F32, kind="ExternalOutput")
    out_ap = d_out.ap()

    with tile.TileContext(nc) as tc:
        with tc.tile_pool(name="w", bufs=1) as wp, \
             tc.tile_pool(name="s", bufs=2) as sp, \
             tc.tile_pool(name="ps", bufs=2, space="PSUM") as pp, \
             tc.tile_pool(name="dr", bufs=2, space="DRAM") as dp:

            # ---- preload weights/constants ----
            wcat = wp.tile([128, 8, 2048], F32)
            woutb = wp.tile([128, 4, VL], BF16)
            woutl = wp.tile([128, 4, VL], BF16)
            bgate = wp.tile([1, 2048], F32)
            bout = wp.tile([B, VL], F32)
            ones = wp.tile([1, B], F32)
            ident = wp.tile([128, 128], F32)
            iota8 = wp.tile([B, 8], F32)
            vbase = wp.tile([B, 1], F32)
            for dst, src in ((wcat, d_wcat), (woutb, d_woutb), (woutl, d_woutl),
                             (bgate, d_bgate), (bout, d_bout), (ones, d_ones),
                             (ident, d_ident), (iota8, d_iota8), (vbase, d_vbase)):
                nc.sync.dma_start(out=dst[:], in_=src.ap()[:])
            # float32r copies for the gate-bias matmuls (4x faster than fp32)
            ones_r = wp.tile([1, B], F32R)
            bgate_r = wp.tile([1, 2048], F32R)
            nc.vector.tensor_copy(out=ones_r[:], in_=ones[:])
            nc.vector.tensor_copy(out=bgate_r[:], in_=bgate[:])

            # ---- initial state ----
            hT = sp.tile([128, 4, B], F32, tag="hT", bufs=2)
            xT = sp.tile([128, 4, B], F32, tag="xT", bufs=2)
            c = sp.tile([B, H], F32, tag="c", bufs=2)
            nc.sync.dma_start(out=hT[:], in_=d_h0t.ap()[:])
            nc.sync.dma_start(out=xT[:], in_=d_x0t.ap()[:])
            nc.vector.memset(c[:], 0.0)

            def h_part(psg_t, hT_src):
                # h @ Whh.T + bias, accumulated into the open gates psum for
                # the NEXT step -- emitted in the exchange tail so the PE has
                # real work (and stays warm) during the collective
                for k in range(4):
                    for n4 in range(4):
                        nc.tensor.matmul(
                            out=psg_t[:, n4 * 512:(n4 + 1) * 512],
                            lhsT=hT_src[:, k, :],
                            rhs=wcat[:, 4 + k, n4 * 512:(n4 + 1) * 512],
                            start=(k == 0), stop=False)
                for n4 in range(4):
                    nc.tensor.matmul(
                        out=psg_t[:, n4 * 512:(n4 + 1) * 512],
                        lhsT=ones_r[:, :],
                        rhs=bgate_r[:, n4 * 512:(n4 + 1) * 512],
                        start=False, stop=False)

            psg = pp.tile([64, 2048], F32, tag="g", bufs=1)
            h_part(psg, hT)

            def heartbeat(ap):
                w = ap.bitcast(BF16)
                p = min(w.shape[0], 128)
                f = min(w.free_size() // (1 if len(w.shape) == 1 else 1), 128)
                nc.tensor.ldweights(weights=w[0:p, 0:min(f, 128)])

            for t in range(T):
                # ---- complete gates: x-part finishes the accumulation the
                # previous step opened with the h-part
                for k in range(4):
                    for n4 in range(4):
                        nc.tensor.matmul(
                            out=psg[:, n4 * 512:(n4 + 1) * 512],
                            lhsT=xT[:, k, :],
                            rhs=wcat[:, k, n4 * 512:(n4 + 1) * 512],
                            start=False, stop=(k == 3))
                gact = []
                for g in range(4):
                    act = sp.tile([B, 512], F32, tag=f"ga{g}", bufs=1)
                    fn = (mybir.ActivationFunctionType.Tanh if g == 2
                          else mybir.ActivationFunctionType.Sigmoid)
                    nc.scalar.activation(act[:], psg[:, g * 512:(g + 1) * 512], fn)
                    gact.append(act)
                si, sf, tg, so = gact
                # c' = sf * c + si * tg ;  h = so * tanh(c')
                t1 = sp.tile([B, H], F32, tag="t1", bufs=1)
                nc.vector.tensor_tensor(out=t1[:], in0=si[:], in1=tg[:],
                                        op=mybir.AluOpType.mult)
                c_new = sp.tile([B, H], F32, tag="c", bufs=2)
                nc.vector.tensor_tensor(out=c_new[:], in0=sf[:], in1=c[:],
                                        op=mybir.AluOpType.mult)
                nc.vector.tensor_tensor(out=c_new[:], in0=c_new[:], in1=t1[:],
                                        op=mybir.AluOpType.add)
                tc_ = sp.tile([B, H], F32, tag="tc", bufs=1)
                nc.scalar.activation(tc_[:], c_new[:],
                                     mybir.ActivationFunctionType.Tanh)
                h_new = sp.tile([B, H], F32, tag="h", bufs=1)
                nc.vector.tensor_tensor(out=h_new[:], in0=so[:], in1=tc_[:],
                                        op=mybir.AluOpType.mult)

                # ---- transpose h -> hT tiles for next matmuls
                hT_new = sp.tile([128, 4, B], F32, tag="hT", bufs=2)
                pst = pp.tile([128, 4 * B], F32, tag="l", bufs=2)
                for k in range(4):
                    nc.tensor.transpose(out=pst[:, k * B:(k + 1) * B],
                                        in_=h_new[:, k * 128:(k + 1) * 128],
                                        identity=ident[0:B, 0:B])
                nc.scalar.activation(hT_new[:].rearrange("p k b -> p (k b)"),
                                     pst[:],
                                     mybir.ActivationFunctionType.Copy)
                # split hT into bf16 head + bf16 residual for 3-term matmul
                hbT = sp.tile([128, 4, B], BF16, tag="hbT", bufs=2)
                hrT = sp.tile([128, 4, B], F32, tag="hrT", bufs=2)
                hlT = sp.tile([128, 4, B], BF16, tag="hlT", bufs=2)
                nc.vector.tensor_copy(out=hbT[:], in_=hT_new[:])
                nc.vector.tensor_tensor(out=hrT[:], in0=hT_new[:], in1=hbT[:],
                                        op=mybir.AluOpType.subtract)
                nc.vector.tensor_copy(out=hlT[:], in_=hrT[:])

                # ---- logits shard:  lg = h @ Wout_loc.T + bout_loc   [B, VL]
                lg = sp.tile([B, VL], F32, tag="lg", bufs=1)
                mq = sp.tile([B, 4], F32, tag="mq", bufs=2)
                for q in range(4):
                    psl = pp.tile([64, GQ], F32, tag="l", bufs=2)
                    terms = [(hbT, woutb), (hbT, woutl), (hlT, woutb)]
                    for k in range(4):
                        for ti, (lh, rh) in enumerate(terms):
                            first = (k == 0 and ti == 0)
                            last = (k == 3 and ti == len(terms) - 1)
                            for n2 in range(2):
                                nc.tensor.matmul(
                                    out=psl[:, n2 * 512:n2 * 512 + 500],
                                    lhsT=lh[:, k, :],
                                    rhs=rh[:, k, q * LQ + n2 * 500:
                                           q * LQ + (n2 + 1) * 500],
                                    start=first, stop=last)
                    # bias add + copy to SBUF, then per-quarter max
                    for n2 in range(2):
                        nc.vector.tensor_tensor(
                            out=lg[:, q * LQ + n2 * 500:q * LQ + (n2 + 1) * 500],
                            in0=psl[:, n2 * 512:n2 * 512 + 500],
                            in1=bout[:, q * LQ + n2 * 500:q * LQ + (n2 + 1) * 500],
                            op=mybir.AluOpType.add)
                    nc.vector.tensor_reduce(
                        out=mq[:, q:q + 1], in_=lg[:, q * LQ:(q + 1) * LQ],
                        axis=mybir.AxisListType.X, op=mybir.AluOpType.max)
                    heartbeat(lg[:, q * LQ:q * LQ + 64])
                    nc.sync.dma_start(out=out_ap[:, t, q * LQ:(q + 1) * LQ],
                                      in_=lg[:, q * LQ:(q + 1) * LQ])

                if stage < 2:
                    hT, xT, c = hT_new, xT, c_new
                    continue
                # ---- local argmax over the shard
                m1 = sp.tile([B, 1], F32, tag="m1", bufs=2)
                nc.vector.tensor_reduce(out=m1[:], in_=mq[:],
                                        axis=mybir.AxisListType.X,
                                        op=mybir.AluOpType.max)
                gm8 = sp.tile([B, 8], F32, tag="gm8", bufs=2)
                nc.vector.tensor_copy(out=gm8[:], in_=m1[:].to_broadcast([B, 8]))
                idx8 = sp.tile([B, 8], U32, tag="idx8", bufs=2)
                nc.vector.max_index(idx8[:], gm8[:], lg[:])
                heartbeat(idx8[:])
                pack = sp.tile([B, 2], F32, tag="pack", bufs=2)
                nc.vector.tensor_copy(out=pack[:, 0:1], in_=m1[:])
                # global idx = local idx + rank * VL
                idxf = sp.tile([B, 1], F32, tag="idxf", bufs=2)
                nc.vector.tensor_copy(out=idxf[:], in_=idx8[:, 0:1])
                nc.vector.tensor_tensor(out=pack[:, 1:2], in0=idxf[:],
                                        in1=vbase[:], op=mybir.AluOpType.add)
                heartbeat(pack[:])

                if stage < 3:
                    hT, xT, c = hT_new, xT, c_new
                    continue
                if t < T - 1:
                    psg = pp.tile([64, 2048], F32, tag="g", bufs=1)
                    h_part(psg, hT_new)

                # ---- exchange (val, idx) with all cores
                cin = dp.tile([B, 2], F32, tag="cin", bufs=2)
                cout = dp.tile([NCORES, B, 2], F32, tag="cout", bufs=2)
                nc.sync.dma_start(out=cin[:], in_=pack[:])
                nc.gpsimd.collective_compute(
                    "AllGather", mybir.AluOpType.bypass,
                    replica_groups=[list(range(NCORES))],
                    ins=[cin[:].opt()], outs=[cout[:].opt()])
                allg = sp.tile([B, NCORES, 2], F32, tag="allg", bufs=2)
                nc.sync.dma_start(out=allg[:],
                                  in_=cout[:].rearrange("r b k -> b r k"))

                # ---- global winner: max value, lowest rank on ties
                vals8 = sp.tile([B, 8], F32, tag="vals8", bufs=2)
                idxs = sp.tile([B, 8], F32, tag="idxs8", bufs=2)
                nc.vector.tensor_copy(
                    out=vals8[:], in_=allg[:, :, 0:1].rearrange("b r k -> b (r k)"))
                heartbeat(vals8[:])
                nc.vector.tensor_copy(
                    out=idxs[:], in_=allg[:, :, 1:2].rearrange("b r k -> b (r k)"))
                wm8 = sp.tile([B, 8], F32, tag="wm8", bufs=2)
                nc.vector.max(wm8[:], vals8[:])
                wr8 = sp.tile([B, 8], U32, tag="wr8", bufs=2)
                nc.vector.max_index(wr8[:], wm8[:], vals8[:])
                wrf = sp.tile([B, 1], F32, tag="wrf", bufs=2)
                nc.vector.tensor_copy(out=wrf[:], in_=wr8[:, 0:1])
                sel = sp.tile([B, 8], F32, tag="sel", bufs=2)
                nc.vector.tensor_tensor(out=sel[:], in0=iota8[:],
                                        in1=wrf[:].to_broadcast([B, 8]),
                                        op=mybir.AluOpType.is_equal)
                nc.vector.tensor_tensor(out=sel[:], in0=sel[:], in1=idxs[:],
                                        op=mybir.AluOpType.mult)
                gidxf = sp.tile([B, 1], F32, tag="gidxf", bufs=2)
                nc.vector.tensor_reduce(out=gidxf[:], in_=sel[:],
                                        axis=mybir.AxisListType.X,
                                        op=mybir.AluOpType.add)
                gidx = sp.tile([B, 1], I32, tag="gidx", bufs=2)
                nc.vector.tensor_copy(out=gidx[:], in_=gidxf[:])
                heartbeat(gidx[:])

                if stage < 4:
                    hT, xT, c = hT_new, xT, c_new
                    continue
                # ---- feedback: x = emb[gidx]  (gather from full local copy)
                x_sb = sp.tile([B, E], F32, tag="x", bufs=2)
                nc.gpsimd.indirect_dma_start(
                    out=x_sb[:], out_offset=None,
                    in_=d_emb.ap()[:],
                    in_offset=bass.IndirectOffsetOnAxis(ap=gidx[:, :1], axis=0))
                heartbeat(x_sb[:, 0:64])
                xT_new = sp.tile([128, 4, B], F32, tag="xT", bufs=2)
                psx = pp.tile([128, 4 * B], F32, tag="l", bufs=2)
                for k in range(4):
                    nc.tensor.transpose(out=psx[:, k * B:(k + 1) * B],
                                        in_=x_sb[:, k * 128:(k + 1) * 128],
                                        identity=ident[0:B, 0:B])
                nc.scalar.activation(xT_new[:].rearrange("p k b -> p (k b)"),
                                     psx[:],
                                     mybir.ActivationFunctionType.Copy)

                hT, xT, c = hT_new, xT_new, c_new

    nc.compile()
    return nc


def _prep_inputs(enc_hs, last_enc_h, bos, emb, W_ih, W_hh, b_ih, b_hh,
                 W_out, b_out):
    emb = np.ascontiguousarray(np.asarray(emb, np.float32))
    wcat_t = np.concatenate([np.asarray(W_ih, np.float32).T,
                             np.asarray(W_hh, np.float32).T], axis=0)  # [1024, 2048]
    wcat = np.ascontiguousarray(wcat_t.reshape(8, 128, 2048).transpose(1, 0, 2))
    bgate = (np.asarray(b_ih, np.float32) + np.asarray(b_hh, np.float32))[None, :]
    h0t = np.ascontiguousarray(
        np.asarray(last_enc_h, np.float32).T.reshape(4, 128, B).transpose(1, 0, 2))
    x0 = np.broadcast_to(np.asarray(bos, np.float32)[None, :], (B, E))
    x0t = np.ascontiguousarray(x0.T.reshape(4, 128, B).transpose(1, 0, 2))
    ones = np.ones((1, B), np.float32)
    ident = np.eye(128, dtype=np.float32)
    iota8 = np.broadcast_to(np.arange(8, dtype=np.float32)[None, :], (B, 8)).copy()
    W_out = np.asarray(W_out, np.float32)
    b_out = np.asarray(b_out, np.float32)

    in_maps = []
    import ml_dtypes
    for r in range(NCORES):
        sh = slice(r * VL, (r + 1) * VL)
        wout_r = np.ascontiguousarray(
            W_out[sh].T.reshape(4, 128, VL).transpose(1, 0, 2))
        woutb_r = wout_r.astype(ml_dtypes.bfloat16)
        woutl_r = (wout_r - woutb_r.astype(np.float32)).astype(ml_dtypes.bfloat16)
        bout_r = np.broadcast_to(b_out[sh][None, :], (B, VL)).copy()
        in_maps.append({
            "wcat": wcat, "bgate": bgate, "woutb": woutb_r, "woutl": woutl_r,
            "bout": bout_r,
            "emb": emb, "h0t": h0t, "x0t": x0t, "ones": ones, "ident": ident,
            "iota8": iota8,
            "myvbase": np.full((B, 1), r * VL, np.float32),
        })
    return in_maps


_CACHE = {}


def _get_nc(T: int, stage: int = 4):
    key = (T, stage)
    if key not in _CACHE:
        _CACHE[key] = build(T, stage)
    return _CACHE[key]


def run(T, trace=False, tmpdir=None, stage=4, **inputs):
    nc = _get_nc(T, stage)
    in_maps = _prep_inputs(**inputs)
    res = bass_utils.run_bass_kernel_spmd(
        nc, in_maps, core_ids=list(range(NCORES)), trace=trace, tmpdir=tmpdir)
    full = np.concatenate(
        [res.results[r]["logits"] for r in range(NCORES)], axis=2)  # [B, T, VL*8]
    dummy = np.zeros((B, T), np.float32)
    return (full, dummy, dummy), res


def kernel(**inputs):
    out, _ = run(64, **inputs)
    return out


# revision 12
# speedup vs baseline: 1.4557x; 1.0492x over previous
"""Greedy autoregressive LSTM decoder on 8 TRN2 NeuronCores.

Strategy: vocab-shard the hidden->vocab projection and the embedding-table
argmax across the 8 cores (4000 vocab rows each, W_out shard resident in
SBUF); replicate the small LSTM weights and redundantly compute the LSTM
cell on every core. Each step every core computes its local logits shard
and local (max, argmax); an 8-core AllGather of the 64 (val, idx) pairs
resolves the global argmax; each core then gathers the winning embedding
rows from its own full copy of the table in DRAM (indirect DMA) and feeds
them back. Logit shards stream to DRAM as they are produced and the host
concatenates along vocab.
"""

import sys

sys.path.insert(0, "/opt/trn_rl_repo")

import numpy as np

import concourse.bacc as bacc
import concourse.bass as bass
import concourse.tile as tile
from concourse import bass_utils, mybir

F32 = mybir.dt.float32
F32R = mybir.dt.float32r
BF16 = mybir.dt.bfloat16
I32 = mybir.dt.int32
U32 = mybir.dt.uint32

B = 64          # batch
E = 512         # embed
H = 512         # hidden
V = 32000       # vocab
NCORES = 8
VL = V // NCORES  # vocab shard per core
GQ = 1024       # gate-psum half width (i,f | g,o)
LQ = 1000       # logits quarter width


def build(T: int, stage: int = 4):
    nc = bacc.Bacc("TRN2", target_bir_lowering=False, debug=False,
                   num_devices=NCORES)

    # ---- kernel I/O ----
    d_wcatb = nc.dram_tensor("wcatb", [128, 8, 2048], BF16, kind="ExternalInput")
    d_wcatl = nc.dram_tensor("wcatl", [128, 8, 2048], BF16, kind="ExternalInput")
    d_bgate = nc.dram_tensor("bgate", [1, 2048], F32, kind="ExternalInput")
    d_woutb = nc.dram_tensor("woutb", [128, 4, VL], BF16, kind="ExternalInput")
    d_woutl = nc.dram_tensor("woutl", [128, 4, VL], BF16, kind="ExternalInput")
    d_bout = nc.dram_tensor("bout", [B, VL], F32, kind="ExternalInput")
    d_emb = nc.dram_tensor("emb", [V, E], F32, kind="ExternalInput")
    d_h0t = nc.dram_tensor("h0t", [128, 4, B], F32, kind="ExternalInput")
    d_x0t = nc.dram_tensor("x0t", [128, 4, B], F32, kind="ExternalInput")
    d_ones = nc.dram_tensor("ones", [1, B], F32, kind="ExternalInput")
    d_ident = nc.dram_tensor("ident", [128, 128], F32, kind="ExternalInput")
    d_iota8 = nc.dram_tensor("iota8", [B, 8], F32, kind="ExternalInput")
    d_vbase = nc.dram_tensor("myvbase", [B, 1], F32, kind="ExternalInput")
    d_out = nc.dram_tensor("logits", [B, T, VL], F32, kind="ExternalOutput")
    out_ap = d_out.ap()

    with tile.TileContext(nc) as tc:
        with tc.tile_pool(name="w", bufs=1) as wp, \
             tc.tile_pool(name="s", bufs=2) as sp, \
             tc.tile_pool(name="ps", bufs=2, space="PSUM") as pp, \
             tc.tile_pool(name="dr", bufs=2, space="DRAM") as dp:

            # ---- preload weights/constants ----
            wcatb = wp.tile([128, 8, 2048], BF16)
            wcatl = wp.tile([128, 8, 2048], BF16)
            woutb = wp.tile([128, 4, VL], BF16)
            woutl = wp.tile([128, 4, VL], BF16)
            bgate = wp.tile([1, 2048], F32)
            bout = wp.tile([B, VL], F32)
            ones = wp.tile([1, B], F32)
            ident = wp.tile([128, 128], F32)
            iota8 = wp.tile([B, 8], F32)
            vbase = wp.tile([B, 1], F32)
            for dst, src in ((wcatb, d_wcatb), (wcatl, d_wcatl),
                             (woutb, d_woutb), (woutl, d_woutl),
                             (bgate, d_bgate), (bout, d_bout), (ones, d_ones),
                             (ident, d_ident), (iota8, d_iota8), (vbase, d_vbase)):
                nc.sync.dma_start(out=dst[:], in_=src.ap()[:])
            # float32r copies for the gate-bias matmuls (4x faster than fp32)
            ones_r = wp.tile([1, B], F32R)
            bgate_r = wp.tile([1, 2048], F32R)
            nc.vector.tensor_copy(out=ones_r[:], in_=ones[:])
            nc.vector.tensor_copy(out=bgate_r[:], in_=bgate[:])

            # ---- initial state ----
            hT = sp.tile([128, 4, B], F32, tag="hT", bufs=2)
            xT = sp.tile([128, 4, B], F32, tag="xT", bufs=2)
            c = sp.tile([B, H], F32, tag="c", bufs=2)
            nc.sync.dma_start(out=hT[:], in_=d_h0t.ap()[:])
            nc.sync.dma_start(out=xT[:], in_=d_x0t.ap()[:])
            nc.vector.memset(c[:], 0.0)

            def bsplit(src, tagp):
                b_ = sp.tile([128, 4, B], BF16, tag=f"{tagp}b", bufs=2)
                r_ = sp.tile([128, 4, B], F32, tag="rsplit", bufs=1)
                l_ = sp.tile([128, 4, B], BF16, tag=f"{tagp}l", bufs=2)
                nc.vector.tensor_copy(out=b_[:], in_=src[:])
                nc.vector.tensor_tensor(out=r_[:], in0=src[:], in1=b_[:],
                                        op=mybir.AluOpType.subtract)
                nc.vector.tensor_copy(out=l_[:], in_=r_[:])
                return b_, l_

            def h_part(psg_t, hT_src):
                # h @ Whh.T + bias, accumulated into the open gates psum for
                # the NEXT step -- emitted in the exchange tail so the PE has
                # real work (and stays warm) during the collective
                hb_, hl_ = hT_src
                for k in range(4):
                    for ti, (lh, rh) in enumerate(((hb_, wcatb), (hb_, wcatl),
                                                   (hl_, wcatb))):
                        for n4 in range(4):
                            nc.tensor.matmul(
                                out=psg_t[:, n4 * 512:(n4 + 1) * 512],
                                lhsT=lh[:, k, :],
                                rhs=rh[:, 4 + k, n4 * 512:(n4 + 1) * 512],
                                start=(k == 0 and ti == 0), stop=False)
                for n4 in range(4):
                    nc.tensor.matmul(
                        out=psg_t[:, n4 * 512:(n4 + 1) * 512],
                        lhsT=ones_r[:, :],
                        rhs=bgate_r[:, n4 * 512:(n4 + 1) * 512],
                        start=False, stop=False)

            hbT, hlT = bsplit(hT, "h")
            xbT, xlT = bsplit(xT, "x")
            psg = pp.tile([64, 2048], F32, tag="g", bufs=1)
            h_part(psg, (hbT, hlT))

            def heartbeat(ap):
                w = ap.bitcast(BF16)
                p = min(w.shape[0], 128)
                f = min(w.free_size() // (1 if len(w.shape) == 1 else 1), 128)
                nc.tensor.ldweights(weights=w[0:p, 0:min(f, 128)])

            for t in range(T):
                # ---- complete gates: x-part finishes the accumulation the
                # previous step opened with the h-part
                for k in range(4):
                    for ti, (lh, rh) in enumerate(((xbT, wcatb), (xbT, wcatl),
                                                   (xlT, wcatb))):
                        for n4 in range(4):
                            nc.tensor.matmul(
                                out=psg[:, n4 * 512:(n4 + 1) * 512],
                                lhsT=lh[:, k, :],
                                rhs=rh[:, k, n4 * 512:(n4 + 1) * 512],
                                start=False,
                                stop=(k == 3 and ti == 2))
                gact = []
                for g in range(4):
                    act = sp.tile([B, 512], F32, tag=f"ga{g}", bufs=1)
                    fn = (mybir.ActivationFunctionType.Tanh if g == 2
                          else mybir.ActivationFunctionType.Sigmoid)
                    nc.scalar.activation(act[:], psg[:, g * 512:(g + 1) * 512], fn)
                    gact.append(act)
                si, sf, tg, so = gact
                # c' = sf * c + si * tg ;  h = so * tanh(c')
                t1 = sp.tile([B, H], F32, tag="t1", bufs=1)
                nc.vector.tensor_tensor(out=t1[:], in0=si[:], in1=tg[:],
                                        op=mybir.AluOpType.mult)
                c_new = sp.tile([B, H], F32, tag="c", bufs=2)
                nc.vector.tensor_tensor(out=c_new[:], in0=sf[:], in1=c[:],
                                        op=mybir.AluOpType.mult)
                nc.vector.tensor_tensor(out=c_new[:], in0=c_new[:], in1=t1[:],
                                        op=mybir.AluOpType.add)
                tc_ = sp.tile([B, H], F32, tag="tc", bufs=1)
                nc.scalar.activation(tc_[:], c_new[:],
                                     mybir.ActivationFunctionType.Tanh)
                h_new = sp.tile([B, H], F32, tag="h", bufs=1)
                nc.vector.tensor_tensor(out=h_new[:], in0=so[:], in1=tc_[:],
                                        op=mybir.AluOpType.mult)

                # ---- transpose h -> hT tiles for next matmuls
                hT_new = sp.tile([128, 4, B], F32, tag="hT", bufs=2)
                pst = pp.tile([128, 4 * B], F32, tag="l", bufs=2)
                for k in range(4):
                    nc.tensor.transpose(out=pst[:, k * B:(k + 1) * B],
                                        in_=h_new[:, k * 128:(k + 1) * 128],
                                        identity=ident[0:B, 0:B])
                nc.scalar.activation(hT_new[:].rearrange("p k b -> p (k b)"),
                                     pst[:],
                                     mybir.ActivationFunctionType.Copy)
                # split hT into bf16 head + bf16 residual for 3-term matmul
                hbT, hlT = bsplit(hT_new, "h")

                # ---- logits shard:  lg = h @ Wout_loc.T + bout_loc   [B, VL]
                lg = sp.tile([B, VL], F32, tag="lg", bufs=1)
                mq = sp.tile([B, 4], F32, tag="mq", bufs=2)
                for q in range(4):
                    psl = pp.tile([64, GQ], F32, tag="l", bufs=2)
                    terms = [(hbT, woutb), (hbT, woutl), (hlT, woutb)]
                    for k in range(4):
                        for ti, (lh, rh) in enumerate(terms):
                            first = (k == 0 and ti == 0)
                            last = (k == 3 and ti == len(terms) - 1)
                            for n2 in range(2):
                                nc.tensor.matmul(
                                    out=psl[:, n2 * 512:n2 * 512 + 500],
                                    lhsT=lh[:, k, :],
                                    rhs=rh[:, k, q * LQ + n2 * 500:
                                           q * LQ + (n2 + 1) * 500],
                                    start=first, stop=last)
                    # bias add + copy to SBUF, then per-quarter max
                    for n2 in range(2):
                        nc.vector.tensor_tensor(
                            out=lg[:, q * LQ + n2 * 500:q * LQ + (n2 + 1) * 500],
                            in0=psl[:, n2 * 512:n2 * 512 + 500],
                            in1=bout[:, q * LQ + n2 * 500:q * LQ + (n2 + 1) * 500],
                            op=mybir.AluOpType.add)
                    nc.vector.tensor_reduce(
                        out=mq[:, q:q + 1], in_=lg[:, q * LQ:(q + 1) * LQ],
                        axis=mybir.AxisListType.X, op=mybir.AluOpType.max)
                    heartbeat(lg[:, q * LQ:q * LQ + 64])
                    nc.sync.dma_start(out=out_ap[:, t, q * LQ:(q + 1) * LQ],
                                      in_=lg[:, q * LQ:(q + 1) * LQ])

                if stage < 2:
                    hT, xT, c = hT_new, xT, c_new
                    continue
                # ---- local argmax over the shard
                m1 = sp.tile([B, 1], F32, tag="m1", bufs=2)
                nc.vector.tensor_reduce(out=m1[:], in_=mq[:],
                                        axis=mybir.AxisListType.X,
                                        op=mybir.AluOpType.max)
                gm8 = sp.tile([B, 8], F32, tag="gm8", bufs=2)
                nc.vector.tensor_copy(out=gm8[:], in_=m1[:].to_broadcast([B, 8]))
                idx8 = sp.tile([B, 8], U32, tag="idx8", bufs=2)
                nc.vector.max_index(idx8[:], gm8[:], lg[:])
                heartbeat(idx8[:])
                pack = sp.tile([B, 2], F32, tag="pack", bufs=2)
                nc.vector.tensor_copy(out=pack[:, 0:1], in_=m1[:])
                # global idx = local idx + rank * VL
                idxf = sp.tile([B, 1], F32, tag="idxf", bufs=2)
                nc.vector.tensor_copy(out=idxf[:], in_=idx8[:, 0:1])
                nc.vector.tensor_tensor(out=pack[:, 1:2], in0=idxf[:],
                                        in1=vbase[:], op=mybir.AluOpType.add)
                heartbeat(pack[:])

                if stage < 3:
                    hT, xT, c = hT_new, xT, c_new
                    continue
                if t < T - 1:
                    psg = pp.tile([64, 2048], F32, tag="g", bufs=1)
                    h_part(psg, (hbT, hlT))

                # ---- exchange (val, idx) with all cores
                cin = dp.tile([B, 2], F32, tag="cin", bufs=2)
                cout = dp.tile([NCORES, B, 2], F32, tag="cout", bufs=2)
                nc.sync.dma_start(out=cin[:], in_=pack[:])
                nc.gpsimd.collective_compute(
                    "AllGather", mybir.AluOpType.bypass,
                    replica_groups=[list(range(NCORES))],
                    ins=[cin[:].opt()], outs=[cout[:].opt()])
                allg = sp.tile([B, NCORES, 2], F32, tag="allg", bufs=2)
                nc.sync.dma_start(out=allg[:],
                                  in_=cout[:].rearrange("r b k -> b r k"))

                # ---- global winner: max value, lowest rank on ties
                vals8 = sp.tile([B, 8], F32, tag="vals8", bufs=2)
                idxs = sp.tile([B, 8], F32, tag="idxs8", bufs=2)
                nc.vector.tensor_copy(
                    out=vals8[:], in_=allg[:, :, 0:1].rearrange("b r k -> b (r k)"))
                heartbeat(vals8[:])
                nc.vector.tensor_copy(
                    out=idxs[:], in_=allg[:, :, 1:2].rearrange("b r k -> b (r k)"))
                wm8 = sp.tile([B, 8], F32, tag="wm8", bufs=2)
                nc.vector.max(wm8[:], vals8[:])
                wr8 = sp.tile([B, 8], U32, tag="wr8", bufs=2)
                nc.vector.max_index(wr8[:], wm8[:], vals8[:])
                wrf = sp.tile([B, 1], F32, tag="wrf", bufs=2)
                nc.vector.tensor_copy(out=wrf[:], in_=wr8[:, 0:1])
                sel = sp.tile([B, 8], F32, tag="sel", bufs=2)
                nc.vector.tensor_tensor(out=sel[:], in0=iota8[:],
                                        in1=wrf[:].to_broadcast([B, 8]),
                                        op=mybir.AluOpType.is_equal)
                nc.vector.tensor_tensor(out=sel[:], in0=sel[:], in1=idxs[:],
                                        op=mybir.AluOpType.mult)
                gidxf = sp.tile([B, 1], F32, tag="gidxf", bufs=2)
                nc.vector.tensor_reduce(out=gidxf[:], in_=sel[:],
                                        axis=mybir.AxisListType.X,
                                        op=mybir.AluOpType.add)
                gidx = sp.tile([B, 1], I32, tag="gidx", bufs=2)
                nc.vector.tensor_copy(out=gidx[:], in_=gidxf[:])
                heartbeat(gidx[:])

                if stage < 4:
                    hT, xT, c = hT_new, xT, c_new
                    continue
                # ---- feedback: x = emb[gidx]  (gather from full local copy)
                x_sb = sp.tile([B, E], F32, tag="x", bufs=2)
                nc.gpsimd.indirect_dma_start(
                    out=x_sb[:], out_offset=None,
                    in_=d_emb.ap()[:],
                    in_offset=bass.IndirectOffsetOnAxis(ap=gidx[:, :1], axis=0))
                heartbeat(x_sb[:, 0:64])
                xT_new = sp.tile([128, 4, B], F32, tag="xT", bufs=2)
                psx = pp.tile([128, 4 * B], F32, tag="l", bufs=2)
                for k in range(4):
                    nc.tensor.transpose(out=psx[:, k * B:(k + 1) * B],
                                        in_=x_sb[:, k * 128:(k + 1) * 128],
                                        identity=ident[0:B, 0:B])
                nc.scalar.activation(xT_new[:].rearrange("p k b -> p (k b)"),
                                     psx[:],
                                     mybir.ActivationFunctionType.Copy)
                xbT, xlT = bsplit(xT_new, "x")

                hT, xT, c = hT_new, xT_new, c_new

    nc.compile()
    return nc


def _prep_inputs(enc_hs, last_enc_h, bos, emb, W_ih, W_hh, b_ih, b_hh,
                 W_out, b_out):
    emb = np.ascontiguousarray(np.asarray(emb, np.float32))
    wcat_t = np.concatenate([np.asarray(W_ih, np.float32).T,
                             np.asarray(W_hh, np.float32).T], axis=0)  # [1024, 2048]
    wcat = np.ascontiguousarray(wcat_t.reshape(8, 128, 2048).transpose(1, 0, 2))
    import ml_dtypes
    wcatb = wcat.astype(ml_dtypes.bfloat16)
    wcatl = (wcat - wcatb.astype(np.float32)).astype(ml_dtypes.bfloat16)
    bgate = (np.asarray(b_ih, np.float32) + np.asarray(b_hh, np.float32))[None, :]
    h0t = np.ascontiguousarray(
        np.asarray(last_enc_h, np.float32).T.reshape(4, 128, B).transpose(1, 0, 2))
    x0 = np.broadcast_to(np.asarray(bos, np.float32)[None, :], (B, E))
    x0t = np.ascontiguousarray(x0.T.reshape(4, 128, B).transpose(1, 0, 2))
    ones = np.ones((1, B), np.float32)
    ident = np.eye(128, dtype=np.float32)
    iota8 = np.broadcast_to(np.arange(8, dtype=np.float32)[None, :], (B, 8)).copy()
    W_out = np.asarray(W_out, np.float32)
    b_out = np.asarray(b_out, np.float32)

    in_maps = []
    for r in range(NCORES):
        sh = slice(r * VL, (r + 1) * VL)
        wout_r = np.ascontiguousarray(
            W_out[sh].T.reshape(4, 128, VL).transpose(1, 0, 2))
        woutb_r = wout_r.astype(ml_dtypes.bfloat16)
        woutl_r = (wout_r - woutb_r.astype(np.float32)).astype(ml_dtypes.bfloat16)
        bout_r = np.broadcast_to(b_out[sh][None, :], (B, VL)).copy()
        in_maps.append({
            "wcatb": wcatb, "wcatl": wcatl, "bgate": bgate,
            "woutb": woutb_r, "woutl": woutl_r,
            "bout": bout_r,
            "emb": emb, "h0t": h0t, "x0t": x0t, "ones": ones, "ident": ident,
            "iota8": iota8,
            "myvbase": np.full((B, 1), r * VL, np.float32),
        })
    return in_maps


_CACHE = {}


def _get_nc(T: int, stage: int = 4):
    key = (T, stage)
    if key not in _CACHE:
        _CACHE[key] = build(T, stage)
    return _CACHE[key]


def run(T, trace=False, tmpdir=None, stage=4, **inputs):
    nc = _get_nc(T, stage)
    in_maps = _prep_inputs(**inputs)
    res = bass_utils.run_bass_kernel_spmd(
        nc, in_maps, core_ids=list(range(NCORES)), trace=trace, tmpdir=tmpdir)
    full = np.concatenate(
        [res.results[r]["logits"] for r in range(NCORES)], axis=2)  # [B, T, VL*8]
    dummy = np.zeros((B, T), np.float32)
    return (full, dummy, dummy), res


def kernel(**inputs):
    out, _ = run(64, **inputs)
    return out


# revision 14
# speedup vs baseline: 1.5084x; 1.0362x over previous
"""Greedy autoregressive LSTM decoder on 8 TRN2 NeuronCores.

Strategy: vocab-shard the hidden->vocab projection and the embedding-table
argmax across the 8 cores (4000 vocab rows each, W_out shard resident in
SBUF); replicate the small LSTM weights and redundantly compute the LSTM
cell on every core. Each step every core computes its local logits shard
and local (max, argmax); an 8-core AllGather of the 64 (val, idx) pairs
resolves the global argmax; each core then gathers the winning embedding
rows from its own full copy of the table in DRAM (indirect DMA) and feeds
them back. Logit shards stream to DRAM as they are produced and the host
concatenates along vocab.
"""

import sys

sys.path.insert(0, "/opt/trn_rl_repo")

import numpy as np

import concourse.bacc as bacc
import concourse.bass as bass
import concourse.tile as tile
from concourse import bass_utils, mybir

F32 = mybir.dt.float32
F32R = mybir.dt.float32r
BF16 = mybir.dt.bfloat16
I32 = mybir.dt.int32
U32 = mybir.dt.uint32

B = 64          # batch
E = 512         # embed
H = 512         # hidden
V = 32000       # vocab
NCORES = 8
VL = V // NCORES  # vocab shard per core
GQ = 1024       # gate-psum half width (i,f | g,o)
LQ = 1000       # logits quarter width


def build(T: int, stage: int = 4):
    nc = bacc.Bacc("TRN2", target_bir_lowering=False, debug=False,
                   num_devices=NCORES)

    # ---- kernel I/O ----
    d_wcatb = nc.dram_tensor("wcatb", [128, 8, 2048], BF16, kind="ExternalInput")
    d_wcatl = nc.dram_tensor("wcatl", [128, 8, 2048], BF16, kind="ExternalInput")
    d_bgate = nc.dram_tensor("bgate", [1, 2048], F32, kind="ExternalInput")
    d_woutb = nc.dram_tensor("woutb", [128, 4, VL], BF16, kind="ExternalInput")
    d_woutl = nc.dram_tensor("woutl", [128, 4, VL], BF16, kind="ExternalInput")
    d_bout = nc.dram_tensor("bout", [B, VL], F32, kind="ExternalInput")
    d_emb = nc.dram_tensor("emb", [V, E], F32, kind="ExternalInput")
    d_h0t = nc.dram_tensor("h0t", [128, 4, B], F32, kind="ExternalInput")
    d_x0t = nc.dram_tensor("x0t", [128, 4, B], F32, kind="ExternalInput")
    d_ones = nc.dram_tensor("ones", [1, B], F32, kind="ExternalInput")
    d_ident = nc.dram_tensor("ident", [128, 128], F32, kind="ExternalInput")
    d_iota8 = nc.dram_tensor("iota8", [B, 8], F32, kind="ExternalInput")
    d_vbase = nc.dram_tensor("myvbase", [B, 1], F32, kind="ExternalInput")
    d_out = nc.dram_tensor("logits", [B, T, VL], F32, kind="ExternalOutput")
    out_ap = d_out.ap()

    with tile.TileContext(nc) as tc:
        with tc.tile_pool(name="w", bufs=1) as wp, \
             tc.tile_pool(name="s", bufs=2) as sp, \
             tc.tile_pool(name="ps", bufs=2, space="PSUM") as pp, \
             tc.tile_pool(name="dr", bufs=2, space="DRAM") as dp:

            # ---- preload weights/constants ----
            wcatb = wp.tile([128, 8, 2048], BF16)
            wcatl = wp.tile([128, 8, 2048], BF16)
            woutb = wp.tile([128, 4, VL], BF16)
            woutl = wp.tile([128, 4, VL], BF16)
            bgate = wp.tile([1, 2048], F32)
            bout = wp.tile([B, VL], F32)
            ones = wp.tile([1, B], F32)
            ident = wp.tile([128, 128], F32)
            iota8 = wp.tile([B, 8], F32)
            vbase = wp.tile([B, 1], F32)
            for dst, src in ((wcatb, d_wcatb), (wcatl, d_wcatl),
                             (woutb, d_woutb), (woutl, d_woutl),
                             (bgate, d_bgate), (bout, d_bout), (ones, d_ones),
                             (ident, d_ident), (iota8, d_iota8), (vbase, d_vbase)):
                nc.sync.dma_start(out=dst[:], in_=src.ap()[:])
            # float32r copies for the gate-bias matmuls (4x faster than fp32)
            ones_r = wp.tile([1, B], F32R)
            bgate_r = wp.tile([1, 2048], F32R)
            nc.vector.tensor_copy(out=ones_r[:], in_=ones[:])
            nc.vector.tensor_copy(out=bgate_r[:], in_=bgate[:])

            # ---- initial state ----
            hT = sp.tile([128, 4, B], F32, tag="hT", bufs=2)
            xT = sp.tile([128, 4, B], F32, tag="xT", bufs=2)
            c = sp.tile([B, H], F32, tag="c", bufs=2)
            nc.sync.dma_start(out=hT[:], in_=d_h0t.ap()[:])
            nc.sync.dma_start(out=xT[:], in_=d_x0t.ap()[:])
            nc.vector.memset(c[:], 0.0)

            def bsplit(src, tagp):
                b_ = sp.tile([128, 4, B], BF16, tag=f"{tagp}b", bufs=2)
                r_ = sp.tile([128, 4, B], F32, tag="rsplit", bufs=1)
                l_ = sp.tile([128, 4, B], BF16, tag=f"{tagp}l", bufs=2)
                nc.vector.tensor_copy(out=b_[:], in_=src[:])
                nc.vector.tensor_tensor(out=r_[:], in0=src[:], in1=b_[:],
                                        op=mybir.AluOpType.subtract)
                nc.vector.tensor_copy(out=l_[:], in_=r_[:])
                return b_, l_

            def h_part(psg_t, hT_src):
                # h @ Whh.T + bias, accumulated into the open gates psum for
                # the NEXT step -- emitted in the exchange tail so the PE has
                # real work (and stays warm) during the collective
                hb_, hl_ = hT_src
                for k in range(4):
                    for ti, (lh, rh) in enumerate(((hb_, wcatb), (hb_, wcatl),
                                                   (hl_, wcatb))):
                        for n4 in range(4):
                            nc.tensor.matmul(
                                out=psg_t[:, n4 * 512:(n4 + 1) * 512],
                                lhsT=lh[:, k, :],
                                rhs=rh[:, 4 + k, n4 * 512:(n4 + 1) * 512],
                                start=(k == 0 and ti == 0), stop=False)
                for n4 in range(4):
                    nc.tensor.matmul(
                        out=psg_t[:, n4 * 512:(n4 + 1) * 512],
                        lhsT=ones_r[:, :],
                        rhs=bgate_r[:, n4 * 512:(n4 + 1) * 512],
                        start=False, stop=False)

            qb4 = wp.tile([B, 4], F32)
            nc.vector.tensor_scalar_mul(qb4[:], iota8[:, 0:4], 1000.0)
            hbT, hlT = bsplit(hT, "h")
            xbT, xlT = bsplit(xT, "x")
            psg = pp.tile([64, 2048], F32, tag="g", bufs=1)
            h_part(psg, (hbT, hlT))

            def heartbeat(ap):
                w = ap.bitcast(BF16)
                p = min(w.shape[0], 128)
                f = min(w.free_size() // (1 if len(w.shape) == 1 else 1), 128)
                nc.tensor.ldweights(weights=w[0:p, 0:min(f, 128)])

            for t in range(T):
                # ---- complete gates: x-part finishes the accumulation the
                # previous step opened with the h-part
                for k in range(4):
                    for ti, (lh, rh) in enumerate(((xbT, wcatb), (xbT, wcatl),
                                                   (xlT, wcatb))):
                        for n4 in range(4):
                            nc.tensor.matmul(
                                out=psg[:, n4 * 512:(n4 + 1) * 512],
                                lhsT=lh[:, k, :],
                                rhs=rh[:, k, n4 * 512:(n4 + 1) * 512],
                                start=False,
                                stop=(k == 3 and ti == 2))
                gact = []
                for g in range(4):
                    act = sp.tile([B, 512], F32, tag=f"ga{g}", bufs=1)
                    fn = (mybir.ActivationFunctionType.Tanh if g == 2
                          else mybir.ActivationFunctionType.Sigmoid)
                    nc.scalar.activation(act[:], psg[:, g * 512:(g + 1) * 512], fn)
                    gact.append(act)
                si, sf, tg, so = gact
                # c' = sf * c + si * tg ;  h = so * tanh(c')
                t1 = sp.tile([B, H], F32, tag="t1", bufs=1)
                nc.vector.tensor_tensor(out=t1[:], in0=si[:], in1=tg[:],
                                        op=mybir.AluOpType.mult)
                c_new = sp.tile([B, H], F32, tag="c", bufs=2)
                nc.vector.tensor_tensor(out=c_new[:], in0=sf[:], in1=c[:],
                                        op=mybir.AluOpType.mult)
                nc.vector.tensor_tensor(out=c_new[:], in0=c_new[:], in1=t1[:],
                                        op=mybir.AluOpType.add)
                tc_ = sp.tile([B, H], F32, tag="tc", bufs=1)
                nc.scalar.activation(tc_[:], c_new[:],
                                     mybir.ActivationFunctionType.Tanh)
                h_new = sp.tile([B, H], F32, tag="h", bufs=1)
                nc.vector.tensor_tensor(out=h_new[:], in0=so[:], in1=tc_[:],
                                        op=mybir.AluOpType.mult)

                # ---- transpose h -> hT tiles for next matmuls
                hT_new = sp.tile([128, 4, B], F32, tag="hT", bufs=2)
                pst = pp.tile([128, 4 * B], F32, tag="l", bufs=2)
                for k in range(4):
                    nc.tensor.transpose(out=pst[:, k * B:(k + 1) * B],
                                        in_=h_new[:, k * 128:(k + 1) * 128],
                                        identity=ident[0:B, 0:B])
                nc.scalar.activation(hT_new[:].rearrange("p k b -> p (k b)"),
                                     pst[:],
                                     mybir.ActivationFunctionType.Copy)
                # split hT into bf16 head + bf16 residual for 3-term matmul
                hbT, hlT = bsplit(hT_new, "h")

                # ---- logits shard:  lg = h @ Wout_loc.T + bout_loc   [B, VL]
                lg = sp.tile([B, VL], F32, tag="lg", bufs=1)
                mq = sp.tile([B, 8], F32, tag="mq", bufs=2)
                nc.vector.memset(mq[:, 4:8], -3.0e38)
                idxcat = sp.tile([B, 4], F32, tag="idxcat", bufs=2)
                for q in range(4):
                    psl = pp.tile([64, GQ], F32, tag="l", bufs=2)
                    terms = [(hbT, woutb), (hbT, woutl), (hlT, woutb)]
                    for k in range(4):
                        for ti, (lh, rh) in enumerate(terms):
                            first = (k == 0 and ti == 0)
                            last = (k == 3 and ti == len(terms) - 1)
                            for n2 in range(2):
                                nc.tensor.matmul(
                                    out=psl[:, n2 * 512:n2 * 512 + 500],
                                    lhsT=lh[:, k, :],
                                    rhs=rh[:, k, q * LQ + n2 * 500:
                                           q * LQ + (n2 + 1) * 500],
                                    start=first, stop=last)
                    # bias add + copy to SBUF, then per-quarter max
                    for n2 in range(2):
                        nc.vector.tensor_tensor(
                            out=lg[:, q * LQ + n2 * 500:q * LQ + (n2 + 1) * 500],
                            in0=psl[:, n2 * 512:n2 * 512 + 500],
                            in1=bout[:, q * LQ + n2 * 500:q * LQ + (n2 + 1) * 500],
                            op=mybir.AluOpType.add)
                    nc.vector.tensor_reduce(
                        out=mq[:, q:q + 1], in_=lg[:, q * LQ:(q + 1) * LQ],
                        axis=mybir.AxisListType.X, op=mybir.AluOpType.max)
                    heartbeat(lg[:, q * LQ:q * LQ + 64])
                    gm8q = sp.tile([B, 8], F32, tag="gm8", bufs=2)
                    nc.vector.tensor_copy(
                        out=gm8q[:], in_=mq[:, q:q + 1].to_broadcast([B, 8]))
                    idx8q = sp.tile([B, 8], U32, tag="idx8", bufs=2)
                    nc.vector.max_index(idx8q[:], gm8q[:],
                                        lg[:, q * LQ:(q + 1) * LQ])
                    nc.vector.tensor_copy(out=idxcat[:, q:q + 1],
                                          in_=idx8q[:, 0:1])
                    nc.sync.dma_start(out=out_ap[:, t, q * LQ:(q + 1) * LQ],
                                      in_=lg[:, q * LQ:(q + 1) * LQ])

                if stage < 2:
                    hT, xT, c = hT_new, xT, c_new
                    continue
                # ---- local argmax: combine the per-quarter results
                m1 = sp.tile([B, 1], F32, tag="m1", bufs=2)
                nc.vector.tensor_reduce(out=m1[:], in_=mq[:],
                                        axis=mybir.AxisListType.X,
                                        op=mybir.AluOpType.max)
                gm8f = sp.tile([B, 8], F32, tag="gm8", bufs=2)
                nc.vector.tensor_copy(out=gm8f[:], in_=m1[:].to_broadcast([B, 8]))
                wq8 = sp.tile([B, 8], U32, tag="wq8", bufs=2)
                nc.vector.max_index(wq8[:], gm8f[:], mq[:])
                heartbeat(wq8[:])
                wqf = sp.tile([B, 1], F32, tag="wqf", bufs=2)
                nc.vector.tensor_copy(out=wqf[:], in_=wq8[:, 0:1])
                sel4 = sp.tile([B, 4], F32, tag="sel4", bufs=2)
                nc.vector.tensor_tensor(out=sel4[:], in0=iota8[:, 0:4],
                                        in1=wqf[:].to_broadcast([B, 4]),
                                        op=mybir.AluOpType.is_equal)
                tmp4 = sp.tile([B, 4], F32, tag="tmp4", bufs=2)
                nc.vector.tensor_tensor(out=tmp4[:], in0=idxcat[:], in1=qb4[:],
                                        op=mybir.AluOpType.add)
                nc.vector.tensor_tensor(out=tmp4[:], in0=tmp4[:], in1=sel4[:],
                                        op=mybir.AluOpType.mult)
                pack = sp.tile([B, 2], F32, tag="pack", bufs=2)
                nc.vector.tensor_copy(out=pack[:, 0:1], in_=m1[:])
                gidxl = sp.tile([B, 1], F32, tag="gidxl", bufs=2)
                nc.vector.tensor_reduce(out=gidxl[:], in_=tmp4[:],
                                        axis=mybir.AxisListType.X,
                                        op=mybir.AluOpType.add)
                nc.vector.tensor_tensor(out=pack[:, 1:2], in0=gidxl[:],
                                        in1=vbase[:], op=mybir.AluOpType.add)
                heartbeat(pack[:])

                if t < T - 1:
                    psg = pp.tile([64, 2048], F32, tag="g", bufs=1)
                    h_part(psg, (hbT, hlT))

                # ---- exchange (val, idx) with all cores
                cin = dp.tile([B, 2], F32, tag="cin", bufs=2)
                cout = dp.tile([NCORES, B, 2], F32, tag="cout", bufs=2)
                nc.sync.dma_start(out=cin[:], in_=pack[:])
                nc.gpsimd.collective_compute(
                    "AllGather", mybir.AluOpType.bypass,
                    replica_groups=[list(range(NCORES))],
                    ins=[cin[:].opt()], outs=[cout[:].opt()])
                allg = sp.tile([B, NCORES, 2], F32, tag="allg", bufs=2)
                nc.sync.dma_start(out=allg[:],
                                  in_=cout[:].rearrange("r b k -> b r k"))

                # ---- global winner: max value, lowest rank on ties
                vals8 = sp.tile([B, 8], F32, tag="vals8", bufs=2)
                idxs = sp.tile([B, 8], F32, tag="idxs8", bufs=2)
                nc.vector.tensor_copy(
                    out=vals8[:], in_=allg[:, :, 0:1].rearrange("b r k -> b (r k)"))
                heartbeat(vals8[:])
                nc.vector.tensor_copy(
                    out=idxs[:], in_=allg[:, :, 1:2].rearrange("b r k -> b (r k)"))
                wm8 = sp.tile([B, 8], F32, tag="wm8", bufs=2)
                nc.vector.max(wm8[:], vals8[:])
                wr8 = sp.tile([B, 8], U32, tag="wr8", bufs=2)
                nc.vector.max_index(wr8[:], wm8[:], vals8[:])
                wrf = sp.tile([B, 1], F32, tag="wrf", bufs=2)
                nc.vector.tensor_copy(out=wrf[:], in_=wr8[:, 0:1])
                sel = sp.tile([B, 8], F32, tag="sel", bufs=2)
                nc.vector.tensor_tensor(out=sel[:], in0=iota8[:],
                                        in1=wrf[:].to_broadcast([B, 8]),
                                        op=mybir.AluOpType.is_equal)
                nc.vector.tensor_tensor(out=sel[:], in0=sel[:], in1=idxs[:],
                                        op=mybir.AluOpType.mult)
                gidxf = sp.tile([B, 1], F32, tag="gidxf", bufs=2)
                nc.vector.tensor_reduce(out=gidxf[:], in_=sel[:],
                                        axis=mybir.AxisListType.X,
                                        op=mybir.AluOpType.add)
                gidx = sp.tile([B, 1], I32, tag="gidx", bufs=2)
                nc.vector.tensor_copy(out=gidx[:], in_=gidxf[:])
                heartbeat(gidx[:])

                if stage < 4:
                    hT, xT, c = hT_new, xT, c_new
                    continue
                # ---- feedback: x = emb[gidx]  (gather from full local copy)
                x_sb = sp.tile([B, E], F32, tag="x", bufs=2)
                nc.gpsimd.indirect_dma_start(
                    out=x_sb[:], out_offset=None,
                    in_=d_emb.ap()[:],
                    in_offset=bass.IndirectOffsetOnAxis(ap=gidx[:, :1], axis=0))
                heartbeat(x_sb[:, 0:64])
                xT_new = sp.tile([128, 4, B], F32, tag="xT", bufs=2)
                psx = pp.tile([128, 4 * B], F32, tag="l", bufs=2)
                for k in range(4):
                    nc.tensor.transpose(out=psx[:, k * B:(k + 1) * B],
                                        in_=x_sb[:, k * 128:(k + 1) * 128],
                                        identity=ident[0:B, 0:B])
                nc.scalar.activation(xT_new[:].rearrange("p k b -> p (k b)"),
                                     psx[:],
                                     mybir.ActivationFunctionType.Copy)
                xbT, xlT = bsplit(xT_new, "x")

                hT, xT, c = hT_new, xT_new, c_new

    nc.compile()
    return nc


def _prep_inputs(enc_hs, last_enc_h, bos, emb, W_ih, W_hh, b_ih, b_hh,
                 W_out, b_out):
    emb = np.ascontiguousarray(np.asarray(emb, np.float32))
    wcat_t = np.concatenate([np.asarray(W_ih, np.float32).T,
                             np.asarray(W_hh, np.float32).T], axis=0)  # [1024, 2048]
    wcat = np.ascontiguousarray(wcat_t.reshape(8, 128, 2048).transpose(1, 0, 2))
    import ml_dtypes
    wcatb = wcat.astype(ml_dtypes.bfloat16)
    wcatl = (wcat - wcatb.astype(np.float32)).astype(ml_dtypes.bfloat16)
    bgate = (np.asarray(b_ih, np.float32) + np.asarray(b_hh, np.float32))[None, :]
    h0t = np.ascontiguousarray(
        np.asarray(last_enc_h, np.float32).T.reshape(4, 128, B).transpose(1, 0, 2))
    x0 = np.broadcast_to(np.asarray(bos, np.float32)[None, :], (B, E))
    x0t = np.ascontiguousarray(x0.T.reshape(4, 128, B).transpose(1, 0, 2))
    ones = np.ones((1, B), np.float32)
    ident = np.eye(128, dtype=np.float32)
    iota8 = np.broadcast_to(np.arange(8, dtype=np.float32)[None, :], (B, 8)).copy()
    W_out = np.asarray(W_out, np.float32)
    b_out = np.asarray(b_out, np.float32)

    in_maps = []
    for r in range(NCORES):
        sh = slice(r * VL, (r + 1) * VL)
        wout_r = np.ascontiguousarray(
            W_out[sh].T.reshape(4, 128, VL).transpose(1, 0, 2))
        woutb_r = wout_r.astype(ml_dtypes.bfloat16)
        woutl_r = (wout_r - woutb_r.astype(np.float32)).astype(ml_dtypes.bfloat16)
        bout_r = np.broadcast_to(b_out[sh][None, :], (B, VL)).copy()
        in_maps.append({
            "wcatb": wcatb, "wcatl": wcatl, "bgate": bgate,
            "woutb": woutb_r, "woutl": woutl_r,
            "bout": bout_r,
            "emb": emb, "h0t": h0t, "x0t": x0t, "ones": ones, "ident": ident,
            "iota8": iota8,
            "myvbase": np.full((B, 1), r * VL, np.float32),
        })
    return in_maps


_CACHE = {}


def _get_nc(T: int, stage: int = 4):
    key = (T, stage)
    if key not in _CACHE:
        _CACHE[key] = build(T, stage)
    return _CACHE[key]


def run(T, trace=False, tmpdir=None, stage=4, **inputs):
    nc = _get_nc(T, stage)
    in_maps = _prep_inputs(**inputs)
    res = bass_utils.run_bass_kernel_spmd(
        nc, in_maps, core_ids=list(range(NCORES)), trace=trace, tmpdir=tmpdir)
    full = np.concatenate(
        [res.results[r]["logits"] for r in range(NCORES)], axis=2)  # [B, T, VL*8]
    dummy = np.zeros((B, T), np.float32)
    return (full, dummy, dummy), res


def kernel(**inputs):
    out, _ = run(64, **inputs)
    return out
